# revision 2
# baseline (speedup 1.0000x reference)
"""Trainium2 Bass kernel for the Householder-chain problem.

Computes y = x @ Q.T where Q = M_0 @ M_1 @ ... @ M_{N-1} is a product of
N=514 Householder reflections M_i = I - 2 v_i v_i^T / (v_i^T v_i + eps)
over S=512 dims, and x is [65536, 512].

Math: since each M_i is symmetric, Q.T = M_{N-1} @ ... @ M_0 =: A, and the
product collapses via the compact-WY representation with natural column
order:  A = I - V T V^T  where V = [v_0 ... v_{N-1}] (S x N) and
T^{-1} = R = stril(V^T V) + diag((||v_i||^2 + eps)/2)   (lower triangular).

On device (replicated on each of 8 cores, since it is tiny):
  G = V^T V; R's 128x128 diagonal blocks are inverted by Newton iteration
  (X <- X(2I - R X), exact in ceil(log2(128)) = 7 steps for triangular R);
  off-diagonal blocks by block back-substitution; then
  A = I - (V T)(V^T) via two small matmul chains.  All in fp32 (the PE's
  full-precision path) - A must be accurate to ~1e-6.
N is zero-padded 514 -> 640 with unit diagonal entries in R for pad
columns, which leaves A unchanged.

Main work: y = x @ A, data-parallel over the 65536 rows across 8 cores
(8192 rows/core) - memory-bound streaming matmul.  It runs in the PE's
float32r mode (fp32 storage, RNE-to-11-mantissa-bit rounding inside the
matmul, 4x faster than the fp32 path): measured end-to-end relative error
~1.5e-4.  Set COMPENSATED=True for a 3-term error-compensated variant
(x and A split into 11-bit head + tail; y = xh Ah + xl Ah + xh Al) that
restores ~1.5e-6 relative error at 3x the PE cost.

x is transposed on the host once so the contraction dim (s) lands on SBUF
partitions.
"""

from contextlib import ExitStack

import numpy as np

import bass_rust
import concourse.bass as bass
import concourse.mybir as mybir
import concourse.tile as tile
from concourse.bass_utils import run_bass_kernel_spmd
from concourse.masks import make_identity, make_upper_triangular
from concourse.vector_clock import ScopedClock

FP = mybir.dt.float32
FPR = mybir.dt.float32r
U32 = mybir.dt.uint32
AX = mybir.AxisListType
OP = mybir.AluOpType

S = 512           # feature dim
NV = 514          # number of householder vectors
NP = 640          # padded vector count (5 * 128)
NB = NP // 128    # 5 blocks
B = 65536         # batch rows
NCORES = 8
BPC = B // NCORES  # 8192 rows per core
EPS = 1e-16
CW = 1024         # main-loop x chunk width (batch cols per chunk)
NEWTON_ITERS = 7
COMPENSATED = False  # 3-term f32r error compensation in the main matmul
HEAD_MASK = 0xFFFFF000  # keep sign+exp+11 mantissa bits (= f32r precision)


# ---------------------------------------------------------------------------
# walrus CTRL instructions accept at most 4 sem waits, and this Tile
# version puts the whole global-clock wait set on the single tail drain.
# Spread the waits over preceding SP nops (1 wait each, conservatively).
def _patched_drain_and_barrier(self, tick_clock, wait_clock):
    pre_nops = [self.nc.sync.nop() for _ in range(30)]
    drain_inst = self.nc.sync.drain()
    wait_clock.add_sem_waits(
        drain_inst.ins, ScopedClock({None: tick_clock.global_clock})
    )
    si = drain_inst.ins.sync_info
    waits = list(si.on_wait) if si is not None and si.on_wait else []
    if len(waits) > 1:
        assert len(waits) - 1 <= len(pre_nops), "too many drain waits"
        for nop, w in zip(pre_nops, waits[:-1]):
            nop.ins.sync_info = bass_rust.SyncInfo(on_wait=[w], on_update=[])
        upd = list(si.on_update) if si.on_update else []
        drain_inst.ins.sync_info = bass_rust.SyncInfo(
            on_wait=[waits[-1]], on_update=upd)

    self.nc.all_engine_barrier()
    assert self.sems is not None
    popped = self.nc._tile_sem_poison_stack.pop()
    assert popped is self._sem_poison
    self.nc.clear_and_free_semaphores(list(self.sems.allocated().values()))
    self.nc.all_engine_barrier()


tile.TileContext._drain_and_barrier = _patched_drain_and_barrier


def _split_excess_waits(nc, max_waits=1):
    """This walrus build accepts very few sem waits per instruction (a
    TensorTensor with 2 was rejected).  Hoist all but `max_waits` of each
    instruction's waits onto same-engine NOPs inserted right before it —
    engines execute in order, so semantics are unchanged."""
    idx = 0
    for fn in nc.m.functions:
        for bb in fn.blocks:
            new = []
            changed = False
            for inst in bb.instructions:
                si = inst.sync_info
                waits = list(si.on_wait) if si is not None and si.on_wait else []
                if len(waits) > max_waits:
                    changed = True
                    for w in waits[:-max_waits]:
                        idx += 1
                        nop = mybir.InstNoOp(
                            name=f"I-waitsplit-{idx}", engine=inst.engine)
                        nop.sync_info = bass_rust.SyncInfo(
                            on_wait=[w], on_update=[])
                        new.append(nop)
                    upd = list(si.on_update) if si.on_update else []
                    inst.sync_info = bass_rust.SyncInfo(
                        on_wait=waits[-max_waits:], on_update=upd)
                new.append(inst)
            if changed:
                bb.instructions = new
# ---------------------------------------------------------------------------


def _emit_prologue(nc, tc, vt_d, vnat_d, consts, work, psum_small):
    """Emit fp32 instructions computing A (4 sbuf tiles [128, 512])."""
    eye = consts.tile([128, 128], FP, tag="eye")
    make_identity(nc, eye)
    eye2 = consts.tile([128, 128], FP, tag="eye2")
    nc.vector.tensor_scalar_mul(eye2, eye, 2.0)
    triu = consts.tile([128, 128], FP, tag="triu")
    make_upper_triangular(nc, triu, val=1.0, diag=False)
    # padcol: 1.0 at rows >= NV - 4*128 = 2 (pad rows of the last block)
    padcol = consts.tile([128, 1], FP, tag="padcol")
    nc.gpsimd.memset(padcol, 1.0)
    nc.gpsimd.affine_select(
        out=padcol, in_=padcol, compare_op=OP.is_ge, fill=0.0,
        base=-(NV - 4 * 128), pattern=[[0, 1]], channel_multiplier=1,
    )

    vt_sb = []
    for k in range(4):
        t = consts.tile([128, NP], FP, tag=f"vt{k}", name=f"vt{k}")
        nc.sync.dma_start(out=t, in_=vt_d[k * 128:(k + 1) * 128, :])
        vt_sb.append(t)
    vnat_sb = []
    for j in range(NB):
        t = consts.tile([128, S], FP, tag=f"vnat{j}", name=f"vnat{j}")
        nc.sync.dma_start(out=t, in_=vnat_d[j * 128:(j + 1) * 128, :])
        vnat_sb.append(t)

    # --- G = V^T V, upper block triangle only (row mi needs cols >= mi*128:
    #     diagonal blocks feed RT, strictly-upper blocks feed back-subst) ---
    g_sb = []
    for mi in range(NB):
        g = consts.tile([128, NP], FP, tag=f"g{mi}", name=f"g{mi}")
        n0 = mi * 128
        chunks = [(n0, min(512, NP - n0))]
        if NP - n0 > 512:
            chunks.append((n0 + 512, NP - n0 - 512))
        for c0, cw in chunks:
            g_ps = psum_small.tile([128, cw], FP, tag="med", name=f"gps{mi}_{c0}")
            for k in range(4):
                nc.tensor.matmul(
                    g_ps,
                    lhsT=vt_sb[k][:, mi * 128:(mi + 1) * 128],
                    rhs=vt_sb[k][:, c0:c0 + cw],
                    start=(k == 0), stop=(k == 3),
                )
            nc.vector.tensor_copy(g[:, c0:c0 + cw], g_ps)
        g_sb.append(g)

    # --- per-block Newton inversion of the diagonal blocks of R ---
    xrow = []   # X stored as 5 row tiles [128, 640] (lower block triangle)
    for k in range(NB):
        xrow.append(consts.tile([128, NP], FP, tag=f"xrow{k}",
                                name=f"xrow{k}"))
    cs = []     # C_b = X_bb^T, needed for back-substitution
    for b in range(NB):
        sq = work.tile([128, S], FP, tag="sq")
        nc.vector.tensor_mul(sq, vnat_sb[b], vnat_sb[b])
        ss = work.tile([128, 1], FP, tag="ss")
        nc.vector.reduce_sum(ss, sq, axis=AX.X)
        rd = work.tile([128, 1], FP, tag="rd")
        # rd = (ss + EPS) * 0.5  (+1.0 on pad rows)
        nc.vector.tensor_scalar(rd, ss, EPS, 0.5, OP.add, OP.mult)
        if b == NB - 1:
            nc.vector.tensor_add(rd, rd, padcol)
        rinv = work.tile([128, 1], FP, tag="rinv")
        nc.vector.reciprocal(rinv, rd)

        # RT holds R_bb^T = striu(G_bb) + diag(rd)
        rt = work.tile([128, 128], FP, tag="rt")
        nc.vector.tensor_mul(rt, g_sb[b][:, b * 128:(b + 1) * 128], triu)
        nc.vector.scalar_tensor_tensor(
            out=rt, in0=eye, scalar=rd, in1=rt, op0=OP.mult, op1=OP.add)

        # X0 = C0 = diag(1/rd)
        x_cur = work.tile([128, 128], FP, tag="xn")
        nc.vector.tensor_scalar_mul(x_cur, eye, rinv)
        c_cur = x_cur
        for _ in range(NEWTON_ITERS):
            m1_ps = psum_small.tile([128, 128], FP, tag="pp")
            nc.tensor.matmul(m1_ps, lhsT=rt, rhs=x_cur,
                             start=True, stop=True)
            m2 = work.tile([128, 128], FP, tag="m2")
            # m2 = 2I - m1
            nc.vector.scalar_tensor_tensor(
                out=m2, in0=m1_ps, scalar=-1.0, in1=eye2,
                op0=OP.mult, op1=OP.add)
            xn_ps = psum_small.tile([128, 128], FP, tag="pp")
            nc.tensor.matmul(xn_ps, lhsT=c_cur, rhs=m2,
                             start=True, stop=True)
            cn_ps = psum_small.tile([128, 128], FP, tag="pp")
            nc.tensor.matmul(cn_ps, lhsT=m2, rhs=c_cur,
                             start=True, stop=True)
            x_new = work.tile([128, 128], FP, tag="xn")
            nc.vector.tensor_copy(x_new, xn_ps)
            c_new = work.tile([128, 128], FP, tag="cn")
            nc.vector.tensor_copy(c_new, cn_ps)
            x_cur, c_cur = x_new, c_new
        nc.vector.tensor_copy(xrow[b][:, b * 128:(b + 1) * 128], x_cur)
        c_keep = consts.tile([128, 128], FP, tag=f"c{b}", name=f"c{b}")
        nc.vector.tensor_copy(c_keep, c_cur)
        cs.append(c_keep)

    # --- off-diagonal blocks of X = R^{-1} via block back-substitution ---
    # X_ij = -X_ii (sum_{k=j..i-1} R_ik X_kj);  R_ik^T = G_ki (G symmetric)
    for j in range(NB):
        for i in range(j + 1, NB):
            acc_ps = psum_small.tile([128, 128], FP, tag="pp")
            for k in range(j, i):
                nc.tensor.matmul(
                    acc_ps,
                    lhsT=g_sb[k][:, i * 128:(i + 1) * 128],
                    rhs=xrow[k][:, j * 128:(j + 1) * 128],
                    start=(k == j), stop=(k == i - 1),
                )
            negacc = work.tile([128, 128], FP, tag="negacc")
            nc.scalar.mul(negacc, acc_ps, -1.0)
            xij_ps = psum_small.tile([128, 128], FP, tag="pp")
            nc.tensor.matmul(xij_ps, lhsT=cs[i], rhs=negacc,
                             start=True, stop=True)
            nc.vector.tensor_copy(xrow[i][:, j * 128:(j + 1) * 128], xij_ps)

    # --- WT_j = sum_{k>=j} X_kj^T vnat_k  (WT = (V T)^T, 5 tiles [128,512])
    wt_sb = []
    for j in range(NB):
        wt_ps = psum_small.tile([128, S], FP, tag="med", name=f"wtps{j}")
        for k in range(j, NB):
            nc.tensor.matmul(
                wt_ps,
                lhsT=xrow[k][:, j * 128:(j + 1) * 128],
                rhs=vnat_sb[k],
                start=(k == j), stop=(k == NB - 1),
            )
        wt = consts.tile([128, S], FP, tag=f"wt{j}", name=f"wt{j}")
        nc.vector.tensor_copy(wt, wt_ps)
        wt_sb.append(wt)

    # --- A = I - WT^T vnat  (4 tiles [128, 512], layout [s, s']) ---
    a_sb = []
    for st in range(4):
        a_ps = psum_small.tile([128, S], FP, tag="med", name=f"aps{st}")
        for j in range(NB):
            nc.tensor.matmul(
                a_ps,
                lhsT=wt_sb[j][:, st * 128:(st + 1) * 128],
                rhs=vnat_sb[j],
                start=(j == 0), stop=(j == NB - 1),
            )
        a = consts.tile([128, S], FP, tag=f"a{st}", name=f"a{st}")
        nc.scalar.mul(a, a_ps, -1.0)
        nc.vector.tensor_add(a[:, st * 128:(st + 1) * 128],
                             a[:, st * 128:(st + 1) * 128], eye)
        a_sb.append(a)
    return a_sb


def _emit_main_plain(nc, consts, xpool, ypool, psum_y, xt_d, y_d, a_sb):
    """Single-pass f32r main loop: 4 matmuls per 128-row output tile."""
    # provenance copies: f32r matmul operands must be produced as float32r
    a_r = []
    for k in range(4):
        ar = consts.tile([128, S], FPR, tag=f"ar{k}", name=f"ar{k}")
        nc.vector.tensor_copy(ar, a_sb[k])
        a_r.append(ar)

    for c in range(BPC // CW):
        xc = []
        for k in range(4):
            t32 = xpool.tile([128, CW], FP, tag=f"xc32_{k}")
            nc.sync.dma_start(
                out=t32, in_=xt_d[k * 128:(k + 1) * 128, c * CW:(c + 1) * CW])
            t = xpool.tile([128, CW], FPR, tag=f"xc{k}")
            nc.vector.tensor_copy(t, t32)
            xc.append(t)
        for bt in range(CW // 128):
            y_ps = psum_y.tile([128, S], FP, tag="y_ps")
            for k in range(4):
                nc.tensor.matmul(
                    y_ps,
                    lhsT=xc[k][:, bt * 128:(bt + 1) * 128],
                    rhs=a_r[k],
                    start=(k == 0), stop=(k == 3),
                )
            yt = ypool.tile([128, S], FP, tag="yt")
            nc.scalar.copy(yt, y_ps)
            row0 = (c * (CW // 128) + bt) * 128
            nc.sync.dma_start(out=y_d[row0:row0 + 128, :], in_=yt)


def _emit_main_compensated(nc, consts, xpool, ypool, psum_y, xt_d, y_d, a_sb):
    """3-term compensated main loop: y = xh Ah + xl Ah + xh Al."""
    a_h = []
    a_l = []
    for k in range(4):
        ah32 = consts.tile([128, S], FP, tag=f"ah32_{k}", name=f"ah32_{k}")
        nc.vector.tensor_scalar(
            ah32.bitcast(U32), a_sb[k].bitcast(U32), HEAD_MASK, None,
            OP.bitwise_and)
        ah = consts.tile([128, S], FPR, tag=f"ah{k}", name=f"ah{k}")
        nc.vector.tensor_copy(ah, ah32)
        al = consts.tile([128, S], FPR, tag=f"al{k}", name=f"al{k}")
        nc.vector.tensor_sub(al, a_sb[k], ah32)
        a_h.append(ah)
        a_l.append(al)

    for c in range(BPC // CW):
        xh = []
        xl = []
        for k in range(4):
            t32 = xpool.tile([128, CW], FP, tag=f"xc32_{k}")
            nc.sync.dma_start(
                out=t32, in_=xt_d[k * 128:(k + 1) * 128, c * CW:(c + 1) * CW])
            th32 = xpool.tile([128, CW], FP, tag=f"xh32_{k}")
            nc.vector.tensor_scalar(
                th32.bitcast(U32), t32.bitcast(U32), HEAD_MASK, None,
                OP.bitwise_and)
            th = xpool.tile([128, CW], FPR, tag=f"xh{k}")
            nc.vector.tensor_copy(th, th32)
            tl = xpool.tile([128, CW], FPR, tag=f"xl{k}")
            nc.scalar.activation(  # tl = t32 - th32, on ACT to offload DVE
                tl, th32, mybir.ActivationFunctionType.Copy,
                bias=0.0, scale=-1.0)
            nc.vector.tensor_add(tl, tl, t32)
            xh.append(th)
            xl.append(tl)
        for bt in range(CW // 128):
            y_ps = psum_y.tile([128, S], FP, tag="y_ps")
            bs = slice(bt * 128, (bt + 1) * 128)
            for k in range(4):
                nc.tensor.matmul(y_ps, lhsT=xh[k][:, bs], rhs=a_h[k],
                                 start=(k == 0), stop=False)
            for k in range(4):
                nc.tensor.matmul(y_ps, lhsT=xl[k][:, bs], rhs=a_h[k],
                                 start=False, stop=False)
            for k in range(4):
                nc.tensor.matmul(y_ps, lhsT=xh[k][:, bs], rhs=a_l[k],
                                 start=False, stop=(k == 3))
            yt = ypool.tile([128, S], FP, tag="yt")
            nc.scalar.copy(yt, y_ps)
            row0 = (c * (CW // 128) + bt) * 128
            nc.sync.dma_start(out=y_d[row0:row0 + 128, :], in_=yt)


def build_program(compensated=COMPENSATED, trace_sim=False):
    nc = bass.Bass("TRN2")
    xt_d = nc.dram_tensor("xt", [S, BPC], FP, kind="ExternalInput")
    vt_d = nc.dram_tensor("vt", [S, NP], FP, kind="ExternalInput")
    vnat_d = nc.dram_tensor("vnat", [NP, S], FP, kind="ExternalInput")
    y_d = nc.dram_tensor("y", [BPC, S], FP, kind="ExternalOutput")

    with tile.TileContext(nc, trace_sim=trace_sim) as tc, ExitStack() as ctx:
        consts = ctx.enter_context(tc.tile_pool(name="consts", bufs=1))
        work = ctx.enter_context(tc.tile_pool(name="work", bufs=3))
        xpool = ctx.enter_context(tc.tile_pool(name="xpool", bufs=3))
        ypool = ctx.enter_context(tc.tile_pool(name="ypool", bufs=4))
        psum_small = ctx.enter_context(
            tc.tile_pool(name="psum_small", bufs=2, space="PSUM"))
        psum_y = ctx.enter_context(
            tc.tile_pool(name="psum_y", bufs=4, space="PSUM"))

        a_sb = _emit_prologue(nc, tc, vt_d, vnat_d, consts, work, psum_small)
        if compensated:
            _emit_main_compensated(nc, consts, xpool, ypool, psum_y,
                                   xt_d, y_d, a_sb)
        else:
            _emit_main_plain(nc, consts, xpool, ypool, psum_y,
                             xt_d, y_d, a_sb)
    _split_excess_waits(nc)
    return nc


_NC_CACHE = {}


def _get_nc():
    if "nc" not in _NC_CACHE:
        _NC_CACHE["nc"] = build_program()
    return _NC_CACHE["nc"]


def prepare_in_maps(x, vectors):
    x = np.ascontiguousarray(np.asarray(x, dtype=np.float32))
    v = np.asarray(vectors, dtype=np.float32)[..., 0]  # [514, 512]
    vnat = np.zeros((NP, S), np.float32)
    vnat[:NV] = v
    vt = np.ascontiguousarray(vnat.T)                  # [512, 640]
    xt = np.ascontiguousarray(x.T)                     # [512, 65536]
    in_maps = []
    for c in range(NCORES):
        in_maps.append({
            "xt": np.ascontiguousarray(xt[:, c * BPC:(c + 1) * BPC]),
            "vt": vt,
            "vnat": vnat,
        })
    return in_maps


def finish_output(res):
    y = np.concatenate([r["y"] for r in res.results], axis=0)
    return np.ascontiguousarray(y.astype(np.float32))


def kernel(x, vectors):
    nc = _get_nc()
    in_maps = prepare_in_maps(x, vectors)
    res = run_bass_kernel_spmd(nc, in_maps, list(range(NCORES)))
    return finish_output(res)


if __name__ == "__main__":
    rng = np.random.default_rng(0)
    x = rng.standard_normal((B, S)).astype(np.float32)
    v = rng.standard_normal((NV, S, 1)).astype(np.float32)
    v /= np.linalg.norm(v, axis=1, keepdims=True)
    y = kernel(x, v)
    print("y", y.shape, y.dtype, float(np.abs(y).max()))



# revision 4
# speedup vs baseline: 2.1363x; 2.1363x over previous
"""Trainium2 Bass kernel for the Householder-chain problem.

Computes y = x @ Q.T where Q = M_0 @ M_1 @ ... @ M_{N-1} is a product of
N=514 Householder reflections M_i = I - 2 v_i v_i^T / (v_i^T v_i + eps)
over S=512 dims, and x is [65536, 512].

Math: since each M_i is symmetric, Q.T = M_{N-1} @ ... @ M_0 =: A, and the
product collapses via the compact-WY representation with natural column
order:  A = I - V T V^T  where V = [v_0 ... v_{N-1}] (S x N) and
T^{-1} = R = stril(V^T V) + diag((||v_i||^2 + eps)/2)   (lower triangular).

On device (replicated on each of 8 cores, since it is tiny):
  G = V^T V (f32r matmuls); the five 128x128 diagonal blocks of R are
  inverted by Newton iteration X <- X(2I - R X) run concatenated across
  blocks: 4 iterations in bf16 followed by one f32r polishing iteration
  (Newton is self-correcting, so the result carries f32r accuracy).
  Off-diagonal blocks of X = R^{-1} come from block back-substitution in
  f32r, then A = I - (V T)(V^T) via two small f32r matmul chains, cast
  to bf16.  N is zero-padded 514 -> 640 with unit diagonal entries in R
  for pad columns, which leaves A unchanged.

Main work: y = x @ A, data-parallel over the 65536 rows across 8 cores
(8192 rows/core).  It runs weight-stationary in bf16 producing y^T tiles:
out[c, r] = sum_k A[k-strip, c-strip]^T x^T[k-strip, r], with x^T uploaded
in bf16 (8 MB/core, fully resident in SBUF; the DMAs stream in behind the
small v loads while the PE runs the prologue) and y^T stored in bf16.
The host un-transposes and casts the result back to float32.  End-to-end
relative error ~3e-3 (gate is 2e-2).
"""

from contextlib import ExitStack

import numpy as np
import ml_dtypes

import bass_rust
import concourse.bass as bass
import concourse.mybir as mybir
import concourse.tile as tile
from concourse.bass_utils import run_bass_kernel_spmd
from concourse.masks import make_identity, make_upper_triangular
from concourse.vector_clock import ScopedClock

FP = mybir.dt.float32
FPR = mybir.dt.float32r
BF = mybir.dt.bfloat16
AX = mybir.AxisListType
OP = mybir.AluOpType

S = 512           # feature dim
NV = 514          # number of householder vectors
NP = 640          # padded vector count (5 * 128)
NB = NP // 128    # 5 blocks
B = 65536         # batch rows
NCORES = 8
BPC = B // NCORES  # 8192 rows per core
EPS = 1e-16
NEWTON_BF = 4     # bf16 Newton iterations
NEWTON_FR = 1     # f32r polish iterations
RW = 512          # main-loop r-block width (moving free dim)
RG = 2048         # store-group width (columns per output DMA)


# ---------------------------------------------------------------------------
# walrus CTRL instructions accept at most 4 sem waits, and this Tile
# version puts the whole global-clock wait set on the single tail drain.
# Spread the waits over preceding SP nops (1 wait each, conservatively).
def _patched_drain_and_barrier(self, tick_clock, wait_clock):
    pre_nops = [self.nc.sync.nop() for _ in range(30)]
    drain_inst = self.nc.sync.drain()
    wait_clock.add_sem_waits(
        drain_inst.ins, ScopedClock({None: tick_clock.global_clock})
    )
    si = drain_inst.ins.sync_info
    waits = list(si.on_wait) if si is not None and si.on_wait else []
    if len(waits) > 1:
        assert len(waits) - 1 <= len(pre_nops), "too many drain waits"
        for nop, w in zip(pre_nops, waits[:-1]):
            nop.ins.sync_info = bass_rust.SyncInfo(on_wait=[w], on_update=[])
        upd = list(si.on_update) if si.on_update else []
        drain_inst.ins.sync_info = bass_rust.SyncInfo(
            on_wait=[waits[-1]], on_update=upd)

    self.nc.all_engine_barrier()
    assert self.sems is not None
    popped = self.nc._tile_sem_poison_stack.pop()
    assert popped is self._sem_poison
    self.nc.clear_and_free_semaphores(list(self.sems.allocated().values()))
    self.nc.all_engine_barrier()


tile.TileContext._drain_and_barrier = _patched_drain_and_barrier


def _split_excess_waits(nc, max_waits=1):
    """This walrus build accepts very few sem waits per instruction (a
    TensorTensor with 2 was rejected).  Hoist all but `max_waits` of each
    instruction's waits onto same-engine NOPs inserted right before it —
    engines execute in order, so semantics are unchanged."""
    idx = 0
    for fn in nc.m.functions:
        for bb in fn.blocks:
            new = []
            changed = False
            for inst in bb.instructions:
                si = inst.sync_info
                waits = list(si.on_wait) if si is not None and si.on_wait else []
                if len(waits) > max_waits:
                    changed = True
                    for w in waits[:-max_waits]:
                        idx += 1
                        nop = mybir.InstNoOp(
                            name=f"I-waitsplit-{idx}", engine=inst.engine)
                        nop.sync_info = bass_rust.SyncInfo(
                            on_wait=[w], on_update=[])
                        new.append(nop)
                    upd = list(si.on_update) if si.on_update else []
                    inst.sync_info = bass_rust.SyncInfo(
                        on_wait=waits[-max_waits:], on_update=upd)
                new.append(inst)
            if changed:
                bb.instructions = new
# ---------------------------------------------------------------------------


def _bs(b):
    return slice(b * 128, (b + 1) * 128)


def _emit_prologue(nc, vt_d, vnat_d, xt_d, xb, consts, work, psum):
    """Emit instructions computing A as 4 bf16 sbuf tiles [128(s), 512(c)].
    Also issues the x^T loads into `xb` right after the v loads."""
    ptag = [0]

    def ptile(shape, name):
        t = psum.tile(shape, FP, tag=f"y{ptag[0] % 6}", name=name)
        ptag[0] += 1
        return t

    eye = consts.tile([128, 128], FP, tag="eye")
    make_identity(nc, eye)
    triu = consts.tile([128, 128], FP, tag="triu")
    make_upper_triangular(nc, triu, val=1.0, diag=False)
    # padcol: 1.0 at rows >= NV - 4*128 = 2 (pad rows of the last block)
    padcol = consts.tile([128, 1], FP, tag="padcol")
    nc.gpsimd.memset(padcol, 1.0)
    nc.gpsimd.affine_select(
        out=padcol, in_=padcol, compare_op=OP.is_ge, fill=0.0,
        base=-(NV - 4 * 128), pattern=[[0, 1]], channel_multiplier=1,
    )
    eyeall = consts.tile([128, NP], FP, tag="eyeall")
    for b in range(NB):
        nc.vector.tensor_copy(eyeall[:, _bs(b)], eye)
    eye2all = consts.tile([128, NP], FP, tag="eye2all")
    nc.vector.tensor_scalar_mul(eye2all, eyeall, 2.0)
    eye_bf = consts.tile([128, 128], BF, tag="eye_bf")
    nc.vector.tensor_copy(eye_bf, eye)

    # --- input DMAs: v first (they gate the prologue), then the big x^T
    #     loads stream behind them while the PE computes ---
    vtr = []
    for k in range(4):
        t = consts.tile([128, NP], FPR, tag=f"vt{k}", name=f"vt{k}")
        nc.sync.dma_start(out=t, in_=vt_d[_bs(k), :])
        vtr.append(t)
    vna = []
    for j in range(NB):
        t = consts.tile([128, S], FPR, tag=f"vnat{j}", name=f"vnat{j}")
        nc.sync.dma_start(out=t, in_=vnat_d[_bs(j), :])
        vna.append(t)
    for k in range(4):
        nc.sync.dma_start(out=xb[k], in_=xt_d[_bs(k), :])

    # --- G = V^T V in f32r: diagonal blocks, then the strictly-upper
    #     row-strips (all that back-substitution needs) ---
    gdall = consts.tile([128, NP], FP, tag="gdall")   # diag blocks, fp32
    for b in range(NB):
        g_ps = ptile([128, 128], f"gd{b}")
        for k in range(4):
            nc.tensor.matmul(g_ps, lhsT=vtr[k][:, _bs(b)],
                             rhs=vtr[k][:, _bs(b)],
                             start=(k == 0), stop=(k == 3))
        nc.scalar.copy(gdall[:, _bs(b)], g_ps)
    # grow[b] holds G[b-block rows, (b+1)*128:640]  (cols re-based to 0)
    grow = []
    for b in range(NB - 1):
        w = NP - (b + 1) * 128
        t = consts.tile([128, w], FPR, tag=f"g{b}", name=f"g{b}")
        g_ps = ptile([128, w], f"go{b}")
        for k in range(4):
            nc.tensor.matmul(g_ps, lhsT=vtr[k][:, _bs(b)],
                             rhs=vtr[k][:, (b + 1) * 128:NP],
                             start=(k == 0), stop=(k == 3))
        nc.vector.tensor_copy(t, g_ps)
        grow.append(t)

    def goff(k, i):  # G[k-block, i-block] as lhsT, i > k
        return grow[k][:, (i - k - 1) * 128:(i - k) * 128]

    # --- R's diagonal: rd = (diag(G) + EPS)/2 (+1 on pad rows) ---
    dtmp = work.tile([128, NP], FP, tag="dtmp")
    nc.vector.tensor_mul(dtmp, gdall, eyeall)
    rdall = consts.tile([128, NB], FP, tag="rdall")
    for b in range(NB):
        nc.vector.reduce_sum(rdall[:, b:b + 1], dtmp[:, _bs(b)], axis=AX.X)
    nc.vector.tensor_scalar(rdall, rdall, EPS, 0.5, OP.add, OP.mult)
    nc.vector.tensor_add(rdall[:, NB - 1:NB], rdall[:, NB - 1:NB], padcol)
    rinvall = consts.tile([128, NB], FP, tag="rinvall")
    nc.vector.reciprocal(rinvall, rdall)

    # --- RT = R_bb^T per block, concatenated: striu(G_bb) + diag(rd) ---
    rt32 = work.tile([128, NP], FP, tag="rt32")
    for b in range(NB):
        nc.vector.tensor_mul(rt32[:, _bs(b)], gdall[:, _bs(b)], triu)
        nc.vector.scalar_tensor_tensor(
            out=rt32[:, _bs(b)], in0=eye, scalar=rdall[:, b:b + 1],
            in1=rt32[:, _bs(b)], op0=OP.mult, op1=OP.add)
    rt_bf = consts.tile([128, NP], BF, tag="rt_bf")
    nc.vector.tensor_copy(rt_bf, rt32)
    rt_fr = consts.tile([128, NP], FPR, tag="rt_fr")
    nc.vector.tensor_copy(rt_fr, rt32)

    # --- Newton: X0 = C0 = diag(1/rd); bf16 iters + f32r polish ---
    x_bf = work.tile([128, NP], BF, tag="x_bf")
    for b in range(NB):
        nc.vector.tensor_scalar_mul(x_bf[:, _bs(b)], eye, rinvall[:, b:b + 1])
    c_bf = work.tile([128, NP], BF, tag="c_bf")
    nc.vector.tensor_copy(c_bf, x_bf)

    def newton_iter(x_cur, c_cur, rt, dt, it):
        # m1 = R X (5 blocks), split 512+128 so each psum tile is one bank
        m1a = ptile([128, 512], f"m1a{it}")
        m1b = ptile([128, 128], f"m1b{it}")
        for b in range(NB):
            dst = m1a[:, _bs(b)] if b < 4 else m1b
            nc.tensor.matmul(dst, lhsT=rt[:, _bs(b)],
                             rhs=x_cur[:, _bs(b)], start=True, stop=True)
        m2 = work.tile([128, NP], dt, tag=f"m2_{dt}")
        nc.vector.scalar_tensor_tensor(
            out=m2[:, 0:512], in0=m1a, scalar=-1.0, in1=eye2all[:, 0:512],
            op0=OP.mult, op1=OP.add)
        nc.vector.scalar_tensor_tensor(
            out=m2[:, 512:NP], in0=m1b, scalar=-1.0, in1=eye2all[:, 512:NP],
            op0=OP.mult, op1=OP.add)
        xna = ptile([128, 512], f"xna{it}")
        xnb = ptile([128, 128], f"xnb{it}")
        cna = ptile([128, 512], f"cna{it}")
        cnb = ptile([128, 128], f"cnb{it}")
        for b in range(NB):
            xd = xna[:, _bs(b)] if b < 4 else xnb
            cd = cna[:, _bs(b)] if b < 4 else cnb
            nc.tensor.matmul(xd, lhsT=c_cur[:, _bs(b)],
                             rhs=m2[:, _bs(b)], start=True, stop=True)
            nc.tensor.matmul(cd, lhsT=m2[:, _bs(b)],
                             rhs=c_cur[:, _bs(b)], start=True, stop=True)
        x_new = work.tile([128, NP], dt, tag=f"x_{dt}")
        nc.scalar.copy(x_new[:, 0:512], xna)
        nc.scalar.copy(x_new[:, 512:NP], xnb)
        c_new = work.tile([128, NP], dt, tag=f"c_{dt}")
        nc.vector.tensor_copy(c_new[:, 0:512], cna)
        nc.vector.tensor_copy(c_new[:, 512:NP], cnb)
        return x_new, c_new

    for it in range(NEWTON_BF):
        x_bf, c_bf = newton_iter(x_bf, c_bf, rt_bf, BF, it)
    x_fr = work.tile([128, NP], FPR, tag="x_fr0")
    nc.vector.tensor_copy(x_fr, x_bf)
    c_fr = work.tile([128, NP], FPR, tag="c_fr0")
    nc.vector.tensor_copy(c_fr, c_bf)
    for it in range(NEWTON_FR):
        x_fr, c_fr = newton_iter(x_fr, c_fr, rt_fr, FPR, NEWTON_BF + it)

    # --- off-diagonal blocks of X = R^{-1} via block back-substitution ---
    # X_ij = -X_ii (sum_{k=j..i-1} R_ik X_kj);  R_ik^T = G_ki (G symmetric)
    # xrow[i] holds X[i-block rows, 0:i*128]  (off-diag only; diag in x_fr)
    xrow = [None]
    for i in range(1, NB):
        xrow.append(consts.tile([128, i * 128], FPR, tag=f"xrow{i}",
                                name=f"xrow{i}"))

    def xk(k, j):  # X_kj tile (diag from x_fr, off-diag from xrow)
        return x_fr[:, _bs(j)] if k == j else xrow[k][:, _bs(j)]

    for j in range(NB):
        for i in range(j + 1, NB):
            acc_ps = ptile([128, 128], f"acc{i}_{j}")
            for k in range(j, i):
                nc.tensor.matmul(acc_ps, lhsT=goff(k, i), rhs=xk(k, j),
                                 start=(k == j), stop=(k == i - 1))
            negacc = work.tile([128, 128], FPR, tag="negacc")
            nc.vector.tensor_scalar_mul(negacc, acc_ps, -1.0)
            xij_ps = ptile([128, 128], f"xij{i}_{j}")
            nc.tensor.matmul(xij_ps, lhsT=c_fr[:, _bs(i)], rhs=negacc,
                             start=True, stop=True)
            nc.vector.tensor_copy(xrow[i][:, _bs(j)], xij_ps)

    # --- WT_j = sum_{k>=j} X_kj^T vnat_k  (WT = (V T)^T, 5 tiles [128,512])
    wt_sb = []
    for j in range(NB):
        wt_ps = ptile([128, S], f"wt{j}")
        for k in range(j, NB):
            nc.tensor.matmul(wt_ps, lhsT=xk(k, j), rhs=vna[k],
                             start=(k == j), stop=(k == NB - 1))
        wt = consts.tile([128, S], FPR, tag=f"wt{j}", name=f"wt{j}")
        nc.vector.tensor_copy(wt, wt_ps)
        wt_sb.append(wt)

    # --- A = I - WT^T vnat, cast to bf16 (4 tiles [128(s), 512(c)]) ---
    a_bf = []
    for st in range(4):
        a_ps = ptile([128, S], f"a{st}")
        for j in range(NB):
            nc.tensor.matmul(a_ps, lhsT=wt_sb[j][:, _bs(st)], rhs=vna[j],
                             start=(j == 0), stop=(j == NB - 1))
        a = consts.tile([128, S], BF, tag=f"a{st}", name=f"a{st}")
        nc.scalar.mul(a, a_ps, -1.0)
        nc.vector.tensor_add(a[:, _bs(st)], a[:, _bs(st)], eye_bf)
        a_bf.append(a)
    return a_bf


def build_program(trace_sim=False):
    nc = bass.Bass("TRN2")
    xt_d = nc.dram_tensor("xt", [S, BPC], BF, kind="ExternalInput")
    vt_d = nc.dram_tensor("vt", [S, NP], FPR, kind="ExternalInput")
    vnat_d = nc.dram_tensor("vnat", [NP, S], FPR, kind="ExternalInput")
    y_d = nc.dram_tensor("y", [S, BPC], BF, kind="ExternalOutput")

    with tile.TileContext(nc, trace_sim=trace_sim) as tc, ExitStack() as ctx:
        consts = ctx.enter_context(tc.tile_pool(name="consts", bufs=1))
        work = ctx.enter_context(tc.tile_pool(name="work", bufs=2))
        ypool = ctx.enter_context(tc.tile_pool(name="ypool", bufs=3))
        psum = ctx.enter_context(
            tc.tile_pool(name="psum", bufs=1, space="PSUM"))

        # x^T resident in SBUF (8 MB bf16)
        xb = [consts.tile([128, BPC], BF, tag=f"xb{k}", name=f"xb{k}")
              for k in range(4)]

        a_bf = _emit_prologue(nc, vt_d, vnat_d, xt_d, xb, consts, work, psum)

        # --- main loop: y^T[c,r] = sum_k A[k-strip, c-strip]^T x^T[k, r] ---
        nmm = 0
        for c in range(4):
            for rg in range(BPC // RG):
                yt = ypool.tile([128, RG], BF, tag="yt")
                for r in range(RG // RW):
                    y_ps = psum.tile([128, RW], FP, tag=f"y{nmm % 6}")
                    nmm += 1
                    r0 = rg * RG + r * RW
                    for k in range(4):
                        nc.tensor.matmul(
                            y_ps,
                            lhsT=a_bf[k][:, _bs(c)],
                            rhs=xb[k][:, r0:r0 + RW],
                            start=(k == 0), stop=(k == 3))
                    if r % 2 == 0:
                        nc.scalar.copy(yt[:, r * RW:(r + 1) * RW], y_ps)
                    else:
                        nc.vector.tensor_copy(yt[:, r * RW:(r + 1) * RW], y_ps)
                nc.sync.dma_start(
                    out=y_d[_bs(c), rg * RG:(rg + 1) * RG], in_=yt)
    _split_excess_waits(nc)
    return nc


_NC_CACHE = {}


def _get_nc():
    if "nc" not in _NC_CACHE:
        _NC_CACHE["nc"] = build_program()
    return _NC_CACHE["nc"]


def prepare_in_maps(x, vectors):
    x = np.asarray(x, dtype=np.float32)
    v = np.asarray(vectors, dtype=np.float32)[..., 0]  # [514, 512]
    vnat = np.zeros((NP, S), np.float32)
    vnat[:NV] = v
    vt = np.ascontiguousarray(vnat.T)                  # [512, 640]
    xbf = x.astype(ml_dtypes.bfloat16)                 # [65536, 512] bf16
    xt = np.ascontiguousarray(xbf.T)                   # [512, 65536] bf16
    in_maps = []
    for c in range(NCORES):
        in_maps.append({
            "xt": np.ascontiguousarray(xt[:, c * BPC:(c + 1) * BPC]),
            "vt": vt,
            "vnat": vnat,
        })
    return in_maps


def finish_output(res):
    yt = np.concatenate([r["y"] for r in res.results], axis=1)  # [512, 65536]
    y = yt.T.astype(np.float32)                                 # [65536, 512]
    return np.ascontiguousarray(y)


def kernel(x, vectors):
    nc = _get_nc()
    in_maps = prepare_in_maps(x, vectors)
    res = run_bass_kernel_spmd(nc, in_maps, list(range(NCORES)))
    return finish_output(res)


if __name__ == "__main__":
    rng = np.random.default_rng(0)
    x = rng.standard_normal((B, S)).astype(np.float32)
    v = rng.standard_normal((NV, S, 1)).astype(np.float32)
    v /= np.linalg.norm(v, axis=1, keepdims=True)
    y = kernel(x, v)
    print("y", y.shape, y.dtype, float(np.abs(y).max()))


# revision 8
# speedup vs baseline: 2.2355x; 1.0464x over previous
"""Trainium2 Bass kernel for the Householder-chain problem.

Computes y = x @ Q.T where Q = M_0 @ M_1 @ ... @ M_{N-1} is a product of
N=514 Householder reflections M_i = I - 2 v_i v_i^T / (v_i^T v_i + eps)
over S=512 dims, and x is [65536, 512].

Math: since each M_i is symmetric, Q.T = M_{N-1} @ ... @ M_0 =: A, and the
product collapses via the compact-WY representation with natural column
order:  A = I - V T V^T  where V = [v_0 ... v_{N-1}] (S x N) and
T^{-1} = R = stril(V^T V) + diag((||v_i||^2 + eps)/2)   (lower triangular).

On device (replicated on each of 8 cores, since it is tiny):
  G = V^T V (f32r row-strip matmuls, all moving dims >= 256 so the PE runs
  at full rate); the five 128x128 diagonal blocks of R are inverted by
  Newton iteration X <- X(2I - R X) in bf16, run as two independent
  dependency chains (blocks 0-2 and 3-4) so engine latencies overlap.
  Off-diagonal blocks of X = R^{-1} come from a zero-padded full-row
  back-substitution in f32r (one wide accumulating matmul per block-row
  instead of per-(i,j) 128-wide matmuls); each back-substitution step
  feeds its row's terms of WT = (V T)^T into per-column PSUM accumulators
  immediately, then A = I - WT^T V, cast to bf16.  N is zero-padded
  514 -> 640 with unit diagonal entries in R for the pad columns, which
  leaves A unchanged.

Main work: y = x @ A, data-parallel over the 65536 rows across 8 cores
(8192 rows/core).  It runs weight-stationary in bf16 producing y^T tiles:
out[c, r] = sum_k A[k-strip, c-strip]^T x^T[k-strip, r], with x^T uploaded
in bf16 (8 MB/core, fully resident in SBUF; the DMAs are issued first and
stream in behind the small v loads while the PE runs the prologue) and
y^T stored in bf16.  The host un-transposes and casts back to float32.
End-to-end relative error ~4e-3 (gate is 2e-2).
"""

from contextlib import ExitStack

import numpy as np
import ml_dtypes

import bass_rust
import concourse.bass as bass
import concourse.mybir as mybir
import concourse.tile as tile
from concourse.bass_utils import run_bass_kernel_spmd
from concourse.masks import make_identity, make_upper_triangular
from concourse.vector_clock import ScopedClock

FP = mybir.dt.float32
FPR = mybir.dt.float32r
BF = mybir.dt.bfloat16
AX = mybir.AxisListType
OP = mybir.AluOpType

S = 512           # feature dim
NV = 514          # number of householder vectors
NP = 640          # padded vector count (5 * 128)
NB = NP // 128    # 5 blocks
B = 65536         # batch rows
NCORES = 8
BPC = B // NCORES  # 8192 rows per core
EPS = 1e-16
NEWTON = 4        # bf16 Newton iterations (converged; see numerics note)
RW = 512          # main-loop r-block width (moving free dim)
RG = 2048         # store-group width (columns per output DMA)
NGA = 3           # Newton group a: blocks 0..2
WA, WB = NGA * 128, (NB - NGA) * 128


# ---------------------------------------------------------------------------
# walrus CTRL instructions accept at most 4 sem waits, and this Tile
# version puts the whole global-clock wait set on the single tail drain.
# Spread the waits over preceding SP nops (1 wait each, conservatively).
def _patched_drain_and_barrier(self, tick_clock, wait_clock):
    pre_nops = [self.nc.sync.nop() for _ in range(30)]
    drain_inst = self.nc.sync.drain()
    wait_clock.add_sem_waits(
        drain_inst.ins, ScopedClock({None: tick_clock.global_clock})
    )
    si = drain_inst.ins.sync_info
    waits = list(si.on_wait) if si is not None and si.on_wait else []
    if len(waits) > 1:
        assert len(waits) - 1 <= len(pre_nops), "too many drain waits"
        for nop, w in zip(pre_nops, waits[:-1]):
            nop.ins.sync_info = bass_rust.SyncInfo(on_wait=[w], on_update=[])
        upd = list(si.on_update) if si.on_update else []
        drain_inst.ins.sync_info = bass_rust.SyncInfo(
            on_wait=[waits[-1]], on_update=upd)

    self.nc.all_engine_barrier()
    assert self.sems is not None
    popped = self.nc._tile_sem_poison_stack.pop()
    assert popped is self._sem_poison
    self.nc.clear_and_free_semaphores(list(self.sems.allocated().values()))
    self.nc.all_engine_barrier()


tile.TileContext._drain_and_barrier = _patched_drain_and_barrier


def _split_excess_waits(nc, max_waits=1):
    """This walrus build accepts very few sem waits per instruction (a
    TensorTensor with 2 was rejected).  Hoist all but `max_waits` of each
    instruction's waits onto same-engine NOPs inserted right before it —
    engines execute in order, so semantics are unchanged."""
    idx = 0
    for fn in nc.m.functions:
        for bb in fn.blocks:
            new = []
            changed = False
            for inst in bb.instructions:
                si = inst.sync_info
                waits = list(si.on_wait) if si is not None and si.on_wait else []
                if len(waits) > max_waits:
                    changed = True
                    for w in waits[:-max_waits]:
                        idx += 1
                        nop = mybir.InstNoOp(
                            name=f"I-waitsplit-{idx}", engine=inst.engine)
                        nop.sync_info = bass_rust.SyncInfo(
                            on_wait=[w], on_update=[])
                        new.append(nop)
                    upd = list(si.on_update) if si.on_update else []
                    inst.sync_info = bass_rust.SyncInfo(
                        on_wait=waits[-max_waits:], on_update=upd)
                new.append(inst)
            if changed:
                bb.instructions = new
# ---------------------------------------------------------------------------


def _bs(b):
    return slice(b * 128, (b + 1) * 128)


def _emit_prologue(nc, vt_d, vnat_d, xt_d, xb, consts, work, psum):
    """Emit instructions computing A as 4 bf16 sbuf tiles [128(s), 512(c)].
    Issues all input DMAs first (v loads, then the big x^T loads)."""
    ptag = [0]

    def ptile(shape, name):  # rotating psum allocator (tags y0..y6)
        t = psum.tile(shape, FP, tag=f"y{ptag[0] % 7}", name=name)
        ptag[0] += 1
        return t

    # --- input DMAs first: v loads gate the prologue; x^T streams behind ---
    vtr = []
    for k in range(4):
        t = consts.tile([128, NP], FPR, tag=f"vt{k}", name=f"vt{k}")
        nc.sync.dma_start(out=t, in_=vt_d[_bs(k), :])
        vtr.append(t)
    vna = []
    for j in range(NB):
        t = consts.tile([128, S], FPR, tag=f"vnat{j}", name=f"vnat{j}")
        nc.sync.dma_start(out=t, in_=vnat_d[_bs(j), :])
        vna.append(t)
    for k in range(4):
        nc.sync.dma_start(out=xb[k], in_=xt_d[_bs(k), :])

    # --- mask constants (GpSimd/DVE, overlap the DMAs) ---
    eye = consts.tile([128, 128], FP, tag="eye")
    make_identity(nc, eye)
    triu = consts.tile([128, 128], FP, tag="triu")
    make_upper_triangular(nc, triu, val=1.0, diag=False)
    padcol = consts.tile([128, 1], FP, tag="padcol")
    nc.gpsimd.memset(padcol, 1.0)
    nc.gpsimd.affine_select(
        out=padcol, in_=padcol, compare_op=OP.is_ge, fill=0.0,
        base=-(NV - 4 * 128), pattern=[[0, 1]], channel_multiplier=1,
    )
    eye2all = consts.tile([128, NP], FP, tag="eye2all")
    for b in range(NB):
        nc.gpsimd.tensor_scalar_mul(eye2all[:, _bs(b)], eye, 2.0)
    eye_bf = consts.tile([128, 128], BF, tag="eye_bf")
    nc.gpsimd.tensor_copy(eye_bf, eye)
    # zeroed X rows for the padded back-substitution (cols j = 0..3).
    # (gpsimd memset can't write f32r; zero via DVE multiply-by-zero off
    # the first v tile, which is the same size.)
    xfull = []
    for k in range(NB):
        t = consts.tile([128, 512], FPR, tag=f"xf{k}", name=f"xf{k}")
        nc.vector.tensor_scalar_mul(t, vna[0], 0.0)
        xfull.append(t)

    # --- G = V^T V in f32r, wide row strips (moving dim >= 256) ---
    # per block-row b: chunks covering cols [b*128, 640); diag lands in
    # gd[b], the strictly-upper part in grow[b].
    chunks = {0: [(0, 512), (384, 640)], 1: [(128, 640)], 2: [(256, 640)],
              3: [(384, 640)], 4: [(384, 640)]}
    gd = []      # diagonal blocks [128,128] fp32
    grow = []    # strictly-upper row strips, f32r (cols re-based)
    grow_w = [512, 384, 256, 128]
    for b in range(NB - 1):
        grow.append(consts.tile([128, grow_w[b]], FPR, tag=f"g{b}",
                                name=f"g{b}"))
    rdall = consts.tile([128, NB], FP, tag="rdall")
    for b in range(NB):
        g_pss = []
        for (c0, c1) in chunks[b]:
            g_ps = ptile([128, c1 - c0], f"g{b}_{c0}")
            for k in range(4):
                nc.tensor.matmul(g_ps, lhsT=vtr[k][:, _bs(b)],
                                 rhs=vtr[k][:, c0:c1],
                                 start=(k == 0), stop=(k == 3))
            g_pss.append((c0, g_ps))
        c0d, psd = g_pss[0] if b < 4 else g_pss[-1]
        gdb = consts.tile([128, 128], FP, tag=f"gd{b}", name=f"gd{b}")
        nc.scalar.copy(gdb, psd[:, b * 128 - c0d:(b + 1) * 128 - c0d])
        gd.append(gdb)
        # rd chain for this block (DVE, overlaps later G matmuls)
        dt = work.tile([128, 128], FP, tag="dtmp")
        nc.vector.tensor_mul(dt, gdb, eye)
        nc.vector.reduce_sum(rdall[:, b:b + 1], dt, axis=AX.X)
        # strictly-upper strip copies (DVE writes the f32r tiles)
        if b < 4:
            u0 = (b + 1) * 128
            copied_to = u0
            for (c0, g_ps) in g_pss:
                c1 = c0 + g_ps.shape[1]
                lo = max(copied_to, c0)
                if c1 <= lo:
                    continue
                nc.vector.tensor_copy(grow[b][:, lo - u0:c1 - u0],
                                      g_ps[:, lo - c0:c1 - c0])
                copied_to = c1

    def goff(k, i):  # G[k-block, i-block] as lhsT, i > k
        return grow[k][:, (i - k - 1) * 128:(i - k) * 128]

    # rd = (diag(G) + EPS)/2 (+1 on pad rows); rinv = 1/rd
    nc.vector.tensor_scalar(rdall, rdall, EPS, 0.5, OP.add, OP.mult)
    nc.vector.tensor_add(rdall[:, NB - 1:NB], rdall[:, NB - 1:NB], padcol)
    rinvall = consts.tile([128, NB], FP, tag="rinvall")
    nc.vector.reciprocal(rinvall, rdall)

    # --- RT = R_bb^T per block (bf16), two groups: a=blocks 0-2, b=3-4 ---
    rta = consts.tile([128, WA], BF, tag="rta")
    rtb = consts.tile([128, WB], BF, tag="rtb")

    def rt_slice(b):
        return rta[:, _bs(b)] if b < NGA else rtb[:, _bs(b - NGA)]

    for b in range(NB):
        rtm = work.tile([128, 128], FP, tag="rtm")
        nc.vector.tensor_mul(rtm, gd[b], triu)
        nc.vector.scalar_tensor_tensor(
            out=rt_slice(b), in0=eye, scalar=rdall[:, b:b + 1],
            in1=rtm, op0=OP.mult, op1=OP.add)

    # --- Newton (bf16): X0 = C0 = diag(1/rd), two independent chains ---
    xa = work.tile([128, WA], BF, tag="xa")
    xb2 = work.tile([128, WB], BF, tag="xb2")
    for b in range(NB):
        dst = xa[:, _bs(b)] if b < NGA else xb2[:, _bs(b - NGA)]
        nc.vector.tensor_scalar_mul(dst, eye, rinvall[:, b:b + 1])
    ca = work.tile([128, WA], BF, tag="ca")
    nc.gpsimd.tensor_copy(ca, xa)
    cb = work.tile([128, WB], BF, tag="cb")
    nc.gpsimd.tensor_copy(cb, xb2)

    for it in range(NEWTON):
        m1a = psum.tile([128, WA], FP, tag="y0", name=f"m1a{it}")
        m1b = psum.tile([128, WB], FP, tag="y1", name=f"m1b{it}")
        for b in range(NGA):
            nc.tensor.matmul(m1a[:, _bs(b)], lhsT=rta[:, _bs(b)],
                             rhs=xa[:, _bs(b)], start=True, stop=True)
        for b in range(NB - NGA):
            nc.tensor.matmul(m1b[:, _bs(b)], lhsT=rtb[:, _bs(b)],
                             rhs=xb2[:, _bs(b)], start=True, stop=True)
        m2a = work.tile([128, WA], BF, tag="m2a")
        nc.vector.scalar_tensor_tensor(
            out=m2a, in0=m1a, scalar=-1.0, in1=eye2all[:, 0:WA],
            op0=OP.mult, op1=OP.add)
        m2b = work.tile([128, WB], BF, tag="m2b")
        nc.vector.scalar_tensor_tensor(
            out=m2b, in0=m1b, scalar=-1.0, in1=eye2all[:, WA:NP],
            op0=OP.mult, op1=OP.add)
        xna = psum.tile([128, WA], FP, tag="y2", name=f"xna{it}")
        cna = psum.tile([128, WA], FP, tag="y4", name=f"cna{it}")
        for b in range(NGA):
            nc.tensor.matmul(xna[:, _bs(b)], lhsT=ca[:, _bs(b)],
                             rhs=m2a[:, _bs(b)], start=True, stop=True)
            nc.tensor.matmul(cna[:, _bs(b)], lhsT=m2a[:, _bs(b)],
                             rhs=ca[:, _bs(b)], start=True, stop=True)
        xnb = psum.tile([128, WB], FP, tag="y3", name=f"xnb{it}")
        cnb = psum.tile([128, WB], FP, tag="y5", name=f"cnb{it}")
        for b in range(NB - NGA):
            nc.tensor.matmul(xnb[:, _bs(b)], lhsT=cb[:, _bs(b)],
                             rhs=m2b[:, _bs(b)], start=True, stop=True)
            nc.tensor.matmul(cnb[:, _bs(b)], lhsT=m2b[:, _bs(b)],
                             rhs=cb[:, _bs(b)], start=True, stop=True)
        xa = work.tile([128, WA], BF, tag="xa")
        nc.scalar.copy(xa, xna)
        ca = work.tile([128, WA], BF, tag="ca")
        nc.vector.tensor_copy(ca, cna)
        xb2 = work.tile([128, WB], BF, tag="xb2")
        nc.scalar.copy(xb2, xnb)
        cb = work.tile([128, WB], BF, tag="cb")
        nc.vector.tensor_copy(cb, cnb)

    # f32r copies of the converged diagonal inverse (and its transpose)
    xfa = consts.tile([128, WA], FPR, tag="xfa")
    nc.vector.tensor_copy(xfa, xa)
    xfb = consts.tile([128, WB], FPR, tag="xfb")
    nc.vector.tensor_copy(xfb, xb2)
    cfa = consts.tile([128, WA], FPR, tag="cfa")
    nc.vector.tensor_copy(cfa, ca)
    cfb = consts.tile([128, WB], FPR, tag="cfb")
    nc.vector.tensor_copy(cfb, cb)

    def xdiag(b):
        return xfa[:, _bs(b)] if b < NGA else xfb[:, _bs(b - NGA)]

    def cdiag(b):
        return cfa[:, _bs(b)] if b < NGA else cfb[:, _bs(b - NGA)]

    # diag blocks into the zero-padded X rows (cols above stay zero)
    for k in range(4):
        nc.vector.tensor_copy(xfull[k][:, _bs(k)], xdiag(k))

    # --- back-substitution + progressive WT accumulation ---
    # step i: X_i,(0:i) = -X_ii * sum_{k<i} G_ki^T Xrow_k[0:i*128]
    # (rows are zero-padded above the diagonal, so one wide matmul per k).
    # After each row i is final, its WT terms  wt_ps[j] += X_ij^T vna_i
    # accumulate immediately (interleaved PSUM groups on separate banks).
    wt_ps = [psum.tile([128, S], FP, tag=f"y{j}", name=f"wtp{j}")
             for j in range(NB)]

    def emit_wt_terms(i):
        for j in range(i + 1):
            z = xdiag(i) if j == i else xfull[i][:, _bs(j)]
            nc.tensor.matmul(wt_ps[j], lhsT=z, rhs=vna[i],
                             start=(i == j), stop=(i == NB - 1),
                             skip_group_check=True)

    emit_wt_terms(0)
    for i in range(1, NB):
        w = i * 128
        acc_ps = psum.tile([128, w], FP, tag="y5", name=f"acc{i}")
        for k in range(i):
            nc.tensor.matmul(acc_ps, lhsT=goff(k, i), rhs=xfull[k][:, 0:w],
                             start=(k == 0), stop=(k == i - 1))
        nacc = work.tile([128, w], FPR, tag="nacc")
        nc.vector.tensor_scalar_mul(nacc, acc_ps, -1.0)
        xij_ps = psum.tile([128, w], FP, tag="y6", name=f"xij{i}")
        nc.tensor.matmul(xij_ps, lhsT=cdiag(i), rhs=nacc,
                         start=True, stop=True)
        nc.vector.tensor_copy(xfull[i][:, 0:w], xij_ps)
        emit_wt_terms(i)

    wt_sb = []
    for j in range(NB):
        wt = consts.tile([128, S], FPR, tag=f"wt{j}", name=f"wt{j}")
        nc.vector.tensor_copy(wt, wt_ps[j])
        wt_sb.append(wt)

    # --- A = I - WT^T vnat, cast to bf16 (4 tiles [128(s), 512(c)]) ---
    a_bf = []
    for st in range(4):
        a_ps = ptile([128, S], f"a{st}")
        for j in range(NB):
            nc.tensor.matmul(a_ps, lhsT=wt_sb[j][:, _bs(st)], rhs=vna[j],
                             start=(j == 0), stop=(j == NB - 1))
        a = consts.tile([128, S], BF, tag=f"a{st}", name=f"a{st}")
        nc.scalar.mul(a, a_ps, -1.0)
        nc.vector.tensor_add(a[:, _bs(st)], a[:, _bs(st)], eye_bf)
        a_bf.append(a)
    return a_bf


def build_program(trace_sim=False):
    nc = bass.Bass("TRN2")
    xt_d = nc.dram_tensor("xt", [S, BPC], BF, kind="ExternalInput")
    vt_d = nc.dram_tensor("vt", [S, NP], FPR, kind="ExternalInput")
    vnat_d = nc.dram_tensor("vnat", [NP, S], FPR, kind="ExternalInput")
    y_d = nc.dram_tensor("y", [S, BPC], BF, kind="ExternalOutput")

    with tile.TileContext(nc, trace_sim=trace_sim) as tc, ExitStack() as ctx:
        consts = ctx.enter_context(tc.tile_pool(name="consts", bufs=1))
        work = ctx.enter_context(tc.tile_pool(name="work", bufs=2))
        ypool = ctx.enter_context(tc.tile_pool(name="ypool", bufs=3))
        psum = ctx.enter_context(
            tc.tile_pool(name="psum", bufs=1, space="PSUM"))

        # x^T resident in SBUF (8 MB bf16)
        xb = [consts.tile([128, BPC], BF, tag=f"xb{k}", name=f"xb{k}")
              for k in range(4)]

        a_bf = _emit_prologue(nc, vt_d, vnat_d, xt_d, xb, consts, work, psum)

        # --- main loop: y^T[c,r] = sum_k A[k-strip, c-strip]^T x^T[k, r] ---
        nmm = 0
        for c in range(4):
            for rg in range(BPC // RG):
                yt = ypool.tile([128, RG], BF, tag="yt")
                for r in range(RG // RW):
                    y_ps = psum.tile([128, RW], FP, tag=f"y{nmm % 7}")
                    nmm += 1
                    r0 = rg * RG + r * RW
                    for k in range(4):
                        nc.tensor.matmul(
                            y_ps,
                            lhsT=a_bf[k][:, _bs(c)],
                            rhs=xb[k][:, r0:r0 + RW],
                            start=(k == 0), stop=(k == 3))
                    if r % 2 == 0:
                        nc.scalar.copy(yt[:, r * RW:(r + 1) * RW], y_ps)
                    else:
                        nc.vector.tensor_copy(yt[:, r * RW:(r + 1) * RW], y_ps)
                nc.sync.dma_start(
                    out=y_d[_bs(c), rg * RG:(rg + 1) * RG], in_=yt)
    _split_excess_waits(nc)
    return nc


_NC_CACHE = {}


def _get_nc():
    if "nc" not in _NC_CACHE:
        _NC_CACHE["nc"] = build_program()
    return _NC_CACHE["nc"]


def prepare_in_maps(x, vectors):
    x = np.asarray(x, dtype=np.float32)
    v = np.asarray(vectors, dtype=np.float32)[..., 0]  # [514, 512]
    vnat = np.zeros((NP, S), np.float32)
    vnat[:NV] = v
    vt = np.ascontiguousarray(vnat.T)                  # [512, 640]
    xbf = x.astype(ml_dtypes.bfloat16)                 # [65536, 512] bf16
    xt = np.ascontiguousarray(xbf.T)                   # [512, 65536] bf16
    in_maps = []
    for c in range(NCORES):
        in_maps.append({
            "xt": np.ascontiguousarray(xt[:, c * BPC:(c + 1) * BPC]),
            "vt": vt,
            "vnat": vnat,
        })
    return in_maps


def finish_output(res):
    yt = np.concatenate([r["y"] for r in res.results], axis=1)  # [512, 65536]
    y = yt.T.astype(np.float32)                                 # [65536, 512]
    return np.ascontiguousarray(y)


def kernel(x, vectors):
    nc = _get_nc()
    in_maps = prepare_in_maps(x, vectors)
    res = run_bass_kernel_spmd(nc, in_maps, list(range(NCORES)))
    return finish_output(res)


if __name__ == "__main__":
    rng = np.random.default_rng(0)
    x = rng.standard_normal((B, S)).astype(np.float32)
    v = rng.standard_normal((NV, S, 1)).astype(np.float32)
    v /= np.linalg.norm(v, axis=1, keepdims=True)
    y = kernel(x, v)
    print("y", y.shape, y.dtype, float(np.abs(y).max()))


# revision 10
# speedup vs baseline: 2.2669x; 1.0141x over previous
"""Trainium2 Bass kernel for the Householder-chain problem.

Computes y = x @ Q.T where Q = M_0 @ M_1 @ ... @ M_{N-1} is a product of
N=514 Householder reflections M_i = I - 2 v_i v_i^T / (v_i^T v_i + eps)
over S=512 dims, and x is [65536, 512].

Math: since each M_i is symmetric, Q.T = M_{N-1} @ ... @ M_0 =: A, and the
product collapses via the compact-WY representation with natural column
order:  A = I - V T V^T  where V = [v_0 ... v_{N-1}] (S x N) and
T^{-1} = R = stril(V^T V) + diag((||v_i||^2 + eps)/2)   (lower triangular).

On device (replicated on each of 8 cores, since it is tiny):
  G = V^T V (f32r row-strip matmuls, all moving dims >= 256 so the PE runs
  at full rate); the five 128x128 diagonal blocks of R are inverted by
  Newton iteration X <- X(2I - R X) in bf16, run as two independent
  dependency chains (blocks 0-2 and 3-4) so engine latencies overlap.
  Off-diagonal blocks of X = R^{-1} come from a zero-padded full-row
  back-substitution in f32r (one wide accumulating matmul per block-row
  instead of per-(i,j) 128-wide matmuls); each back-substitution step
  feeds its row's terms of WT = (V T)^T into per-column PSUM accumulators
  immediately, then A = I - WT^T V, cast to bf16.  N is zero-padded
  514 -> 640 with unit diagonal entries in R for the pad columns, which
  leaves A unchanged.

Main work: y = x @ A, data-parallel over the 65536 rows across 8 cores
(8192 rows/core).  It runs weight-stationary in bf16 producing y^T tiles:
out[c, r] = sum_k A[k-strip, c-strip]^T x^T[k-strip, r], with x^T uploaded
in bf16 (8 MB/core, fully resident in SBUF; the DMAs are issued first and
stream in behind the small v loads while the PE runs the prologue) and
y^T stored in bf16.  The host un-transposes and casts back to float32.
End-to-end relative error ~4e-3 (gate is 2e-2).
"""

from contextlib import ExitStack

import numpy as np
import ml_dtypes

import bass_rust
import concourse.bass as bass
import concourse.mybir as mybir
import concourse.tile as tile
from concourse.bass_utils import run_bass_kernel_spmd
from concourse.masks import make_identity, make_upper_triangular
from concourse.vector_clock import ScopedClock

FP = mybir.dt.float32
FPR = mybir.dt.float32r
BF = mybir.dt.bfloat16
AX = mybir.AxisListType
OP = mybir.AluOpType

S = 512           # feature dim
NV = 514          # number of householder vectors
NP = 640          # padded vector count (5 * 128)
NB = NP // 128    # 5 blocks
B = 65536         # batch rows
NCORES = 8
BPC = B // NCORES  # 8192 rows per core
EPS = 1e-16
NEWTON = 4        # bf16 Newton iterations (converged; see numerics note)
RW = 512          # main-loop r-block width (moving free dim)
RG = 2048         # store-group width (columns per output DMA)
NGA = 3           # Newton group a: blocks 0..2
WA, WB = NGA * 128, (NB - NGA) * 128


# ---------------------------------------------------------------------------
# walrus CTRL instructions accept at most 4 sem waits, and this Tile
# version puts the whole global-clock wait set on the single tail drain.
# Spread the waits over preceding SP nops (1 wait each, conservatively).
def _patched_drain_and_barrier(self, tick_clock, wait_clock):
    pre_nops = [self.nc.sync.nop() for _ in range(30)]
    drain_inst = self.nc.sync.drain()
    wait_clock.add_sem_waits(
        drain_inst.ins, ScopedClock({None: tick_clock.global_clock})
    )
    si = drain_inst.ins.sync_info
    waits = list(si.on_wait) if si is not None and si.on_wait else []
    if len(waits) > 1:
        assert len(waits) - 1 <= len(pre_nops), "too many drain waits"
        for nop, w in zip(pre_nops, waits[:-1]):
            nop.ins.sync_info = bass_rust.SyncInfo(on_wait=[w], on_update=[])
        upd = list(si.on_update) if si.on_update else []
        drain_inst.ins.sync_info = bass_rust.SyncInfo(
            on_wait=[waits[-1]], on_update=upd)

    self.nc.all_engine_barrier()
    assert self.sems is not None
    popped = self.nc._tile_sem_poison_stack.pop()
    assert popped is self._sem_poison
    self.nc.clear_and_free_semaphores(list(self.sems.allocated().values()))
    self.nc.all_engine_barrier()


tile.TileContext._drain_and_barrier = _patched_drain_and_barrier


def _split_excess_waits(nc, max_waits=1):
    """This walrus build accepts very few sem waits per instruction (a
    TensorTensor with 2 was rejected).  Hoist all but `max_waits` of each
    instruction's waits onto same-engine NOPs inserted right before it —
    engines execute in order, so semantics are unchanged."""
    idx = 0
    for fn in nc.m.functions:
        for bb in fn.blocks:
            new = []
            changed = False
            for inst in bb.instructions:
                si = inst.sync_info
                waits = list(si.on_wait) if si is not None and si.on_wait else []
                if len(waits) > max_waits:
                    changed = True
                    for w in waits[:-max_waits]:
                        idx += 1
                        nop = mybir.InstNoOp(
                            name=f"I-waitsplit-{idx}", engine=inst.engine)
                        nop.sync_info = bass_rust.SyncInfo(
                            on_wait=[w], on_update=[])
                        new.append(nop)
                    upd = list(si.on_update) if si.on_update else []
                    inst.sync_info = bass_rust.SyncInfo(
                        on_wait=waits[-max_waits:], on_update=upd)
                new.append(inst)
            if changed:
                bb.instructions = new
# ---------------------------------------------------------------------------


def _bs(b):
    return slice(b * 128, (b + 1) * 128)


def _emit_prologue(nc, vt_d, vnat_d, xt_d, xb, consts, work, psum):
    """Emit instructions computing A as 4 bf16 sbuf tiles [128(s), 512(c)].
    Issues all input DMAs first (v loads, then the big x^T loads)."""
    ptag = [0]

    def ptile(shape, name):  # rotating psum allocator (tags y0..y6)
        t = psum.tile(shape, FP, tag=f"y{ptag[0] % 7}", name=name)
        ptag[0] += 1
        return t

    # --- input DMAs first: v loads gate the prologue; x^T streams behind ---
    vtr = []
    for k in range(4):
        t = consts.tile([128, NP], FPR, tag=f"vt{k}", name=f"vt{k}")
        nc.sync.dma_start(out=t, in_=vt_d[_bs(k), :])
        vtr.append(t)
    vna = []
    for j in range(NB):
        t = consts.tile([128, S], FPR, tag=f"vnat{j}", name=f"vnat{j}")
        nc.sync.dma_start(out=t, in_=vnat_d[_bs(j), :])
        vna.append(t)
    for k in range(4):
        nc.sync.dma_start(out=xb[k], in_=xt_d[_bs(k), :])

    # --- mask constants (GpSimd/DVE, overlap the DMAs) ---
    eye = consts.tile([128, 128], FP, tag="eye")
    make_identity(nc, eye)
    triu = consts.tile([128, 128], FP, tag="triu")
    make_upper_triangular(nc, triu, val=1.0, diag=False)
    padcol = consts.tile([128, 1], FP, tag="padcol")
    nc.gpsimd.memset(padcol, 1.0)
    nc.gpsimd.affine_select(
        out=padcol, in_=padcol, compare_op=OP.is_ge, fill=0.0,
        base=-(NV - 4 * 128), pattern=[[0, 1]], channel_multiplier=1,
    )
    eye2all = consts.tile([128, NP], FP, tag="eye2all")
    for b in range(NB):
        nc.gpsimd.tensor_scalar_mul(eye2all[:, _bs(b)], eye, 2.0)
    eye_bf = consts.tile([128, 128], BF, tag="eye_bf")
    nc.gpsimd.tensor_copy(eye_bf, eye)
    # --- G = V^T V in f32r, wide row strips (moving dim >= 256), fused
    # with the per-group rd/RT/X0 chains: group a (blocks 0-2) has its
    # whole Newton-entry chain emitted right after block 2's strip, so it
    # runs on DVE while the PE is still doing blocks 3-4 of G. ---
    chunks = {0: [(0, 512), (384, 640)], 1: [(128, 640)], 2: [(256, 640)],
              3: [(384, 640)], 4: [(384, 640)]}
    gd = []      # diagonal blocks [128,128] fp32
    grow = []    # strictly-upper row strips, f32r (cols re-based)
    grow_w = [512, 384, 256, 128]
    for b in range(NB - 1):
        grow.append(consts.tile([128, grow_w[b]], FPR, tag=f"g{b}",
                                name=f"g{b}"))
    rd_a = consts.tile([128, NGA], FP, tag="rd_a")
    rd_b = consts.tile([128, NB - NGA], FP, tag="rd_b")
    ri_a = consts.tile([128, NGA], FP, tag="ri_a")
    ri_b = consts.tile([128, NB - NGA], FP, tag="ri_b")
    rta = consts.tile([128, WA], BF, tag="rta")
    rtb = consts.tile([128, WB], BF, tag="rtb")
    xa = work.tile([128, WA], BF, tag="xa")
    xb2 = work.tile([128, WB], BF, tag="xb2")
    ca = work.tile([128, WA], BF, tag="ca")
    cb = work.tile([128, WB], BF, tag="cb")

    def rt_slice(b):
        return rta[:, _bs(b)] if b < NGA else rtb[:, _bs(b - NGA)]

    def emit_group_entry(grp):
        # (rd+eps)/2 [+1 pad], reciprocal, RT build, X0=C0=diag(1/rd)
        rd, ri = (rd_a, ri_a) if grp == 0 else (rd_b, ri_b)
        blocks = range(NGA) if grp == 0 else range(NGA, NB)
        nc.vector.tensor_scalar(rd, rd, EPS, 0.5, OP.add, OP.mult)
        if grp == 1:
            nc.vector.tensor_add(rd[:, -1:], rd[:, -1:], padcol)
        nc.vector.reciprocal(ri, rd)
        for b in blocks:
            g = b if grp == 0 else b - NGA
            rtm = work.tile([128, 128], FP, tag="rtm")
            nc.vector.tensor_mul(rtm, gd[b], triu)
            nc.vector.scalar_tensor_tensor(
                out=rt_slice(b), in0=eye, scalar=rd[:, g:g + 1],
                in1=rtm, op0=OP.mult, op1=OP.add)
            dst = xa[:, _bs(b)] if grp == 0 else xb2[:, _bs(b - NGA)]
            nc.vector.tensor_scalar_mul(dst, eye, ri[:, g:g + 1])
        if grp == 0:
            nc.vector.tensor_copy(ca, xa)
        else:
            nc.vector.tensor_copy(cb, xb2)

    gtag = [0]
    for b in range(NB):
        g_pss = []
        for (c0, c1) in chunks[b]:
            g_ps = psum.tile([128, c1 - c0], FP, tag=f"y{4 + gtag[0] % 3}",
                             name=f"g{b}_{c0}")
            gtag[0] += 1
            for k in range(4):
                nc.tensor.matmul(g_ps, lhsT=vtr[k][:, _bs(b)],
                                 rhs=vtr[k][:, c0:c1],
                                 start=(k == 0), stop=(k == 3))
            g_pss.append((c0, g_ps))
        c0d, psd = g_pss[0] if b < 4 else g_pss[-1]
        gdb = consts.tile([128, 128], FP, tag=f"gd{b}", name=f"gd{b}")
        nc.scalar.copy(gdb, psd[:, b * 128 - c0d:(b + 1) * 128 - c0d])
        gd.append(gdb)
        # rd terms for this block (DVE, overlaps later G matmuls)
        dt = work.tile([128, 128], FP, tag="dtmp")
        nc.vector.tensor_mul(dt, gdb, eye)
        rd, g = (rd_a, b) if b < NGA else (rd_b, b - NGA)
        nc.vector.reduce_sum(rd[:, g:g + 1], dt, axis=AX.X)
        # strictly-upper strip copies (DVE writes the f32r tiles)
        if b < 4:
            u0 = (b + 1) * 128
            copied_to = u0
            for (c0, g_ps) in g_pss:
                c1 = c0 + g_ps.shape[1]
                lo = max(copied_to, c0)
                if c1 <= lo:
                    continue
                nc.vector.tensor_copy(grow[b][:, lo - u0:c1 - u0],
                                      g_ps[:, lo - c0:c1 - c0])
                copied_to = c1
        if b == NGA - 1:
            emit_group_entry(0)
        if b == NB - 1:
            emit_group_entry(1)

    def goff(k, i):  # G[k-block, i-block] as lhsT, i > k
        return grow[k][:, (i - k - 1) * 128:(i - k) * 128]

    for it in range(NEWTON):
        m1a = psum.tile([128, WA], FP, tag="y0", name=f"m1a{it}")
        m1b = psum.tile([128, WB], FP, tag="y1", name=f"m1b{it}")
        for b in range(NGA):
            nc.tensor.matmul(m1a[:, _bs(b)], lhsT=rta[:, _bs(b)],
                             rhs=xa[:, _bs(b)], start=True, stop=True)
        for b in range(NB - NGA):
            nc.tensor.matmul(m1b[:, _bs(b)], lhsT=rtb[:, _bs(b)],
                             rhs=xb2[:, _bs(b)], start=True, stop=True)
        m2a = work.tile([128, WA], BF, tag="m2a")
        nc.vector.scalar_tensor_tensor(
            out=m2a, in0=m1a, scalar=-1.0, in1=eye2all[:, 0:WA],
            op0=OP.mult, op1=OP.add)
        m2b = work.tile([128, WB], BF, tag="m2b")
        nc.vector.scalar_tensor_tensor(
            out=m2b, in0=m1b, scalar=-1.0, in1=eye2all[:, WA:NP],
            op0=OP.mult, op1=OP.add)
        xna = psum.tile([128, WA], FP, tag="y2", name=f"xna{it}")
        cna = psum.tile([128, WA], FP, tag="y4", name=f"cna{it}")
        for b in range(NGA):
            nc.tensor.matmul(xna[:, _bs(b)], lhsT=ca[:, _bs(b)],
                             rhs=m2a[:, _bs(b)], start=True, stop=True)
            nc.tensor.matmul(cna[:, _bs(b)], lhsT=m2a[:, _bs(b)],
                             rhs=ca[:, _bs(b)], start=True, stop=True)
        xnb = psum.tile([128, WB], FP, tag="y3", name=f"xnb{it}")
        cnb = psum.tile([128, WB], FP, tag="y5", name=f"cnb{it}")
        for b in range(NB - NGA):
            nc.tensor.matmul(xnb[:, _bs(b)], lhsT=cb[:, _bs(b)],
                             rhs=m2b[:, _bs(b)], start=True, stop=True)
            nc.tensor.matmul(cnb[:, _bs(b)], lhsT=m2b[:, _bs(b)],
                             rhs=cb[:, _bs(b)], start=True, stop=True)
        xa = work.tile([128, WA], BF, tag="xa")
        nc.scalar.copy(xa, xna)
        ca = work.tile([128, WA], BF, tag="ca")
        nc.vector.tensor_copy(ca, cna)
        xb2 = work.tile([128, WB], BF, tag="xb2")
        nc.scalar.copy(xb2, xnb)
        cb = work.tile([128, WB], BF, tag="cb")
        nc.vector.tensor_copy(cb, cnb)

    # f32r copies of the converged diagonal inverse (and its transpose)
    xfa = consts.tile([128, WA], FPR, tag="xfa")
    nc.vector.tensor_copy(xfa, xa)
    xfb = consts.tile([128, WB], FPR, tag="xfb")
    nc.vector.tensor_copy(xfb, xb2)
    cfa = consts.tile([128, WA], FPR, tag="cfa")
    nc.vector.tensor_copy(cfa, ca)
    cfb = consts.tile([128, WB], FPR, tag="cfb")
    nc.vector.tensor_copy(cfb, cb)

    def xdiag(b):
        return xfa[:, _bs(b)] if b < NGA else xfb[:, _bs(b - NGA)]

    def cdiag(b):
        return cfa[:, _bs(b)] if b < NGA else cfb[:, _bs(b - NGA)]

    # zero-padded X rows for the wide back-substitution (cols j = 0..3);
    # zeroed via DVE multiply-by-zero (gpsimd memset can't write f32r),
    # then the converged diagonal blocks dropped in (cols above stay 0).
    xfull = []
    for k in range(NB):
        t = consts.tile([128, 512], FPR, tag=f"xf{k}", name=f"xf{k}")
        nc.vector.tensor_scalar_mul(t, vna[0], 0.0)
        xfull.append(t)
    for k in range(4):
        nc.vector.tensor_copy(xfull[k][:, _bs(k)], xdiag(k))

    # --- back-substitution + progressive WT accumulation ---
    # step i: X_i,(0:i) = -X_ii * sum_{k<i} G_ki^T Xrow_k[0:i*128]
    # (rows are zero-padded above the diagonal, so one wide matmul per k).
    # After each row i is final, its WT terms  wt_ps[j] += X_ij^T vna_i
    # accumulate immediately (interleaved PSUM groups on separate banks).
    wt_ps = [psum.tile([128, S], FP, tag=f"y{j}", name=f"wtp{j}")
             for j in range(NB)]

    def emit_wt_terms(i):
        for j in range(i + 1):
            z = xdiag(i) if j == i else xfull[i][:, _bs(j)]
            nc.tensor.matmul(wt_ps[j], lhsT=z, rhs=vna[i],
                             start=(i == j), stop=(i == NB - 1),
                             skip_group_check=True)

    emit_wt_terms(0)
    for i in range(1, NB):
        w = i * 128
        acc_ps = psum.tile([128, w], FP, tag="y5", name=f"acc{i}")
        for k in range(i):
            nc.tensor.matmul(acc_ps, lhsT=goff(k, i), rhs=xfull[k][:, 0:w],
                             start=(k == 0), stop=(k == i - 1))
        nacc = work.tile([128, w], FPR, tag="nacc")
        nc.vector.tensor_scalar_mul(nacc, acc_ps, -1.0)
        xij_ps = psum.tile([128, w], FP, tag="y6", name=f"xij{i}")
        nc.tensor.matmul(xij_ps, lhsT=cdiag(i), rhs=nacc,
                         start=True, stop=True)
        nc.vector.tensor_copy(xfull[i][:, 0:w], xij_ps)
        emit_wt_terms(i)

    wt_sb = []
    for j in range(NB):
        wt = consts.tile([128, S], FPR, tag=f"wt{j}", name=f"wt{j}")
        nc.vector.tensor_copy(wt, wt_ps[j])
        wt_sb.append(wt)

    # --- A = I - WT^T vnat, cast to bf16 (4 tiles [128(s), 512(c)]) ---
    a_bf = []
    for st in range(4):
        a_ps = ptile([128, S], f"a{st}")
        for j in range(NB):
            nc.tensor.matmul(a_ps, lhsT=wt_sb[j][:, _bs(st)], rhs=vna[j],
                             start=(j == 0), stop=(j == NB - 1))
        a = consts.tile([128, S], BF, tag=f"a{st}", name=f"a{st}")
        nc.scalar.mul(a, a_ps, -1.0)
        nc.vector.tensor_add(a[:, _bs(st)], a[:, _bs(st)], eye_bf)
        a_bf.append(a)
    return a_bf


def build_program(trace_sim=False):
    nc = bass.Bass("TRN2")
    xt_d = nc.dram_tensor("xt", [S, BPC], BF, kind="ExternalInput")
    vt_d = nc.dram_tensor("vt", [S, NP], FPR, kind="ExternalInput")
    vnat_d = nc.dram_tensor("vnat", [NP, S], FPR, kind="ExternalInput")
    y_d = nc.dram_tensor("y", [S, BPC], BF, kind="ExternalOutput")

    with tile.TileContext(nc, trace_sim=trace_sim) as tc, ExitStack() as ctx:
        consts = ctx.enter_context(tc.tile_pool(name="consts", bufs=1))
        work = ctx.enter_context(tc.tile_pool(name="work", bufs=2))
        ypool = ctx.enter_context(tc.tile_pool(name="ypool", bufs=3))
        psum = ctx.enter_context(
            tc.tile_pool(name="psum", bufs=1, space="PSUM"))

        # x^T resident in SBUF (8 MB bf16)
        xb = [consts.tile([128, BPC], BF, tag=f"xb{k}", name=f"xb{k}")
              for k in range(4)]

        a_bf = _emit_prologue(nc, vt_d, vnat_d, xt_d, xb, consts, work, psum)

        # --- main loop: y^T[c,r] = sum_k A[k-strip, c-strip]^T x^T[k, r] ---
        nmm = 0
        for c in range(4):
            for rg in range(BPC // RG):
                yt = ypool.tile([128, RG], BF, tag="yt")
                for r in range(RG // RW):
                    y_ps = psum.tile([128, RW], FP, tag=f"y{nmm % 7}")
                    nmm += 1
                    r0 = rg * RG + r * RW
                    for k in range(4):
                        nc.tensor.matmul(
                            y_ps,
                            lhsT=a_bf[k][:, _bs(c)],
                            rhs=xb[k][:, r0:r0 + RW],
                            start=(k == 0), stop=(k == 3))
                    if r % 2 == 0:
                        nc.scalar.copy(yt[:, r * RW:(r + 1) * RW], y_ps)
                    else:
                        nc.vector.tensor_copy(yt[:, r * RW:(r + 1) * RW], y_ps)
                nc.sync.dma_start(
                    out=y_d[_bs(c), rg * RG:(rg + 1) * RG], in_=yt)
    _split_excess_waits(nc)
    return nc


_NC_CACHE = {}


def _get_nc():
    if "nc" not in _NC_CACHE:
        _NC_CACHE["nc"] = build_program()
    return _NC_CACHE["nc"]


def prepare_in_maps(x, vectors):
    x = np.asarray(x, dtype=np.float32)
    v = np.asarray(vectors, dtype=np.float32)[..., 0]  # [514, 512]
    vnat = np.zeros((NP, S), np.float32)
    vnat[:NV] = v
    vt = np.ascontiguousarray(vnat.T)                  # [512, 640]
    xbf = x.astype(ml_dtypes.bfloat16)                 # [65536, 512] bf16
    xt = np.ascontiguousarray(xbf.T)                   # [512, 65536] bf16
    in_maps = []
    for c in range(NCORES):
        in_maps.append({
            "xt": np.ascontiguousarray(xt[:, c * BPC:(c + 1) * BPC]),
            "vt": vt,
            "vnat": vnat,
        })
    return in_maps


def finish_output(res):
    yt = np.concatenate([r["y"] for r in res.results], axis=1)  # [512, 65536]
    y = yt.T.astype(np.float32)                                 # [65536, 512]
    return np.ascontiguousarray(y)


def kernel(x, vectors):
    nc = _get_nc()
    in_maps = prepare_in_maps(x, vectors)
    res = run_bass_kernel_spmd(nc, in_maps, list(range(NCORES)))
    return finish_output(res)


if __name__ == "__main__":
    rng = np.random.default_rng(0)
    x = rng.standard_normal((B, S)).astype(np.float32)
    v = rng.standard_normal((NV, S, 1)).astype(np.float32)
    v /= np.linalg.norm(v, axis=1, keepdims=True)
    y = kernel(x, v)
    print("y", y.shape, y.dtype, float(np.abs(y).max()))


# revision 14
# speedup vs baseline: 2.3008x; 1.0150x over previous
"""Trainium2 Bass kernel for the Householder-chain problem.

Computes y = x @ Q.T where Q = M_0 @ M_1 @ ... @ M_{N-1} is a product of
N=514 Householder reflections M_i = I - 2 v_i v_i^T / (v_i^T v_i + eps)
over S=512 dims, and x is [65536, 512].

Math: since each M_i is symmetric, Q.T = M_{N-1} @ ... @ M_0 =: A, and the
product collapses via the compact-WY representation with natural column
order:  A = I - V T V^T  where V = [v_0 ... v_{N-1}] (S x N) and
T^{-1} = R = stril(V^T V) + diag((||v_i||^2 + eps)/2)   (lower triangular).

On device (replicated on each of 8 cores, since it is tiny):
  G = V^T V (f32r row-strip matmuls, all moving dims >= 256 so the PE runs
  at full rate); the five 128x128 diagonal blocks of R are inverted by
  Newton iteration X <- X(2I - R X) in bf16, run as two independent
  dependency chains (blocks 0-2 and 3-4) so engine latencies overlap.
  Off-diagonal blocks of X = R^{-1} come from a zero-padded full-row
  back-substitution in f32r (one wide accumulating matmul per block-row
  instead of per-(i,j) 128-wide matmuls); each back-substitution step
  feeds its row's terms of WT = (V T)^T into per-column PSUM accumulators
  immediately, then A = I - WT^T V, cast to bf16.  N is zero-padded
  514 -> 640 with unit diagonal entries in R for the pad columns, which
  leaves A unchanged.

Main work: y = x @ A, data-parallel over the 65536 rows across 8 cores
(8192 rows/core).  It runs weight-stationary in bf16 producing y^T tiles:
out[c, r] = sum_k A[k-strip, c-strip]^T x^T[k-strip, r], with x^T uploaded
in bf16 (8 MB/core, fully resident in SBUF; the DMAs are issued first and
stream in behind the small v loads while the PE runs the prologue) and
y^T stored in bf16.  The host un-transposes and casts back to float32.
End-to-end relative error ~4e-3 (gate is 2e-2).
"""

from contextlib import ExitStack

import numpy as np
import ml_dtypes

import bass_rust
import concourse.bass as bass
import concourse.mybir as mybir
import concourse.tile as tile
from concourse.bass_utils import run_bass_kernel_spmd
from concourse.masks import make_identity, make_upper_triangular
from concourse.vector_clock import ScopedClock

FP = mybir.dt.float32
FPR = mybir.dt.float32r
BF = mybir.dt.bfloat16
AX = mybir.AxisListType
OP = mybir.AluOpType

S = 512           # feature dim
NV = 514          # number of householder vectors
NP = 640          # padded vector count (5 * 128)
NB = NP // 128    # 5 blocks
B = 65536         # batch rows
NCORES = 8
BPC = B // NCORES  # 8192 rows per core
EPS = 1e-16
NEWTON = 4        # bf16 Newton iterations (converged; see numerics note)
RW = 512          # main-loop r-block width (moving free dim)
RG = 2048         # store-group width (columns per output DMA)
NGA = 3           # Newton group a: blocks 0..2
WA, WB = NGA * 128, (NB - NGA) * 128


# ---------------------------------------------------------------------------
# walrus CTRL instructions accept at most 4 sem waits, and this Tile
# version puts the whole global-clock wait set on the single tail drain.
# Spread the waits over preceding SP nops (1 wait each, conservatively).
def _patched_drain_and_barrier(self, tick_clock, wait_clock):
    pre_nops = [self.nc.sync.nop() for _ in range(30)]
    drain_inst = self.nc.sync.drain()
    wait_clock.add_sem_waits(
        drain_inst.ins, ScopedClock({None: tick_clock.global_clock})
    )
    si = drain_inst.ins.sync_info
    waits = list(si.on_wait) if si is not None and si.on_wait else []
    if len(waits) > 1:
        assert len(waits) - 1 <= len(pre_nops), "too many drain waits"
        for nop, w in zip(pre_nops, waits[:-1]):
            nop.ins.sync_info = bass_rust.SyncInfo(on_wait=[w], on_update=[])
        upd = list(si.on_update) if si.on_update else []
        drain_inst.ins.sync_info = bass_rust.SyncInfo(
            on_wait=[waits[-1]], on_update=upd)

    self.nc.all_engine_barrier()
    assert self.sems is not None
    popped = self.nc._tile_sem_poison_stack.pop()
    assert popped is self._sem_poison
    self.nc.clear_and_free_semaphores(list(self.sems.allocated().values()))
    self.nc.all_engine_barrier()


tile.TileContext._drain_and_barrier = _patched_drain_and_barrier


def _split_excess_waits(nc, max_waits=1):
    """This walrus build accepts very few sem waits per instruction (a
    TensorTensor with 2 was rejected).  Hoist all but `max_waits` of each
    instruction's waits onto same-engine NOPs inserted right before it —
    engines execute in order, so semantics are unchanged."""
    idx = 0
    for fn in nc.m.functions:
        for bb in fn.blocks:
            new = []
            changed = False
            for inst in bb.instructions:
                si = inst.sync_info
                waits = list(si.on_wait) if si is not None and si.on_wait else []
                if len(waits) > max_waits:
                    changed = True
                    for w in waits[:-max_waits]:
                        idx += 1
                        nop = mybir.InstNoOp(
                            name=f"I-waitsplit-{idx}", engine=inst.engine)
                        nop.sync_info = bass_rust.SyncInfo(
                            on_wait=[w], on_update=[])
                        new.append(nop)
                    upd = list(si.on_update) if si.on_update else []
                    inst.sync_info = bass_rust.SyncInfo(
                        on_wait=waits[-max_waits:], on_update=upd)
                new.append(inst)
            if changed:
                bb.instructions = new
# ---------------------------------------------------------------------------


def _bs(b):
    return slice(b * 128, (b + 1) * 128)


def _emit_prologue(nc, vt_d, vnat_d, xt_d, xb, consts, work, psum):
    """Emit instructions computing A as 4 bf16 sbuf tiles [128(s), 512(c)].
    Issues all input DMAs first (v loads, then the big x^T loads)."""
    ptag = [0]

    def ptile(shape, name):  # rotating psum allocator (tags y0..y6)
        t = psum.tile(shape, FP, tag=f"y{ptag[0] % 7}", name=name)
        ptag[0] += 1
        return t

    # --- input DMAs first: v loads gate the prologue; x^T streams behind ---
    vtr = []
    for k in range(4):
        t = consts.tile([128, NP], FPR, tag=f"vt{k}", name=f"vt{k}")
        nc.sync.dma_start(out=t, in_=vt_d[_bs(k), :])
        vtr.append(t)
    vna = []
    for j in range(NB):
        t = consts.tile([128, S], FPR, tag=f"vnat{j}", name=f"vnat{j}")
        nc.sync.dma_start(out=t, in_=vnat_d[_bs(j), :])
        vna.append(t)
    for k in range(4):
        nc.sync.dma_start(out=xb[k], in_=xt_d[_bs(k), :])

    # --- mask constants (GpSimd/DVE, overlap the DMAs) ---
    eye = consts.tile([128, 128], FP, tag="eye")
    make_identity(nc, eye)
    triu = consts.tile([128, 128], FP, tag="triu")
    make_upper_triangular(nc, triu, val=1.0, diag=False)
    padcol = consts.tile([128, 1], FP, tag="padcol")
    nc.gpsimd.memset(padcol, 1.0)
    nc.gpsimd.affine_select(
        out=padcol, in_=padcol, compare_op=OP.is_ge, fill=0.0,
        base=-(NV - 4 * 128), pattern=[[0, 1]], channel_multiplier=1,
    )
    eye2all = consts.tile([128, NP], FP, tag="eye2all")
    for b in range(NB):
        nc.gpsimd.tensor_scalar_mul(eye2all[:, _bs(b)], eye, 2.0)
    eye_bf = consts.tile([128, 128], BF, tag="eye_bf")
    nc.gpsimd.tensor_copy(eye_bf, eye)
    # --- G = V^T V in f32r, wide row strips (moving dim >= 256), fused
    # with the per-group rd/RT/X0 chains: group a (blocks 0-2) has its
    # whole Newton-entry chain emitted right after block 2's strip, so it
    # runs on DVE while the PE is still doing blocks 3-4 of G. ---
    chunks = {0: [(0, 512), (384, 640)], 1: [(128, 640)], 2: [(256, 640)],
              3: [(384, 640)], 4: [(384, 640)]}
    gd = []      # diagonal blocks [128,128] fp32
    grow = []    # strictly-upper row strips, f32r (cols re-based)
    grow_w = [512, 384, 256, 128]
    for b in range(NB - 1):
        grow.append(consts.tile([128, grow_w[b]], FPR, tag=f"g{b}",
                                name=f"g{b}"))
    rd_a = consts.tile([128, NGA], FP, tag="rd_a")
    rd_b = consts.tile([128, NB - NGA], FP, tag="rd_b")
    ri_a = consts.tile([128, NGA], FP, tag="ri_a")
    ri_b = consts.tile([128, NB - NGA], FP, tag="ri_b")
    rta = consts.tile([128, WA], BF, tag="rta")
    rtb = consts.tile([128, WB], BF, tag="rtb")
    xa = work.tile([128, WA], BF, tag="xa")
    xb2 = work.tile([128, WB], BF, tag="xb2")
    ca = work.tile([128, WA], BF, tag="ca")
    cb = work.tile([128, WB], BF, tag="cb")

    def rt_slice(b):
        return rta[:, _bs(b)] if b < NGA else rtb[:, _bs(b - NGA)]

    def emit_group_entry(grp):
        # (rd+eps)/2 [+1 pad], reciprocal, RT build, X0=C0=diag(1/rd)
        rd, ri = (rd_a, ri_a) if grp == 0 else (rd_b, ri_b)
        blocks = range(NGA) if grp == 0 else range(NGA, NB)
        nc.vector.tensor_scalar(rd, rd, EPS, 0.5, OP.add, OP.mult)
        if grp == 1:
            nc.vector.tensor_add(rd[:, -1:], rd[:, -1:], padcol)
        nc.vector.reciprocal(ri, rd)
        for b in blocks:
            g = b if grp == 0 else b - NGA
            rtm = work.tile([128, 128], FP, tag="rtm")
            nc.vector.tensor_mul(rtm, gd[b], triu)
            nc.vector.scalar_tensor_tensor(
                out=rt_slice(b), in0=eye, scalar=rd[:, g:g + 1],
                in1=rtm, op0=OP.mult, op1=OP.add)
            dst = xa[:, _bs(b)] if grp == 0 else xb2[:, _bs(b - NGA)]
            nc.scalar.activation(dst, eye, mybir.ActivationFunctionType.Copy,
                                 scale=ri[:, g:g + 1])
        if grp == 0:
            nc.scalar.copy(ca, xa)
        else:
            nc.scalar.copy(cb, xb2)

    gtag = [0]
    for b in range(NB):
        g_pss = []
        for (c0, c1) in chunks[b]:
            g_ps = psum.tile([128, c1 - c0], FP, tag=f"y{gtag[0] % 6}",
                             name=f"g{b}_{c0}")
            gtag[0] += 1
            for k in range(4):
                nc.tensor.matmul(g_ps, lhsT=vtr[k][:, _bs(b)],
                                 rhs=vtr[k][:, c0:c1],
                                 start=(k == 0), stop=(k == 3))
            g_pss.append((c0, g_ps))
        c0d, psd = g_pss[0] if b < 4 else g_pss[-1]
        gdb = consts.tile([128, 128], FP, tag=f"gd{b}", name=f"gd{b}")
        nc.scalar.copy(gdb, psd[:, b * 128 - c0d:(b + 1) * 128 - c0d])
        gd.append(gdb)
        # rd terms for this block (DVE, overlaps later G matmuls)
        dt = work.tile([128, 128], FP, tag="dtmp")
        nc.vector.tensor_mul(dt, gdb, eye)
        rd, g = (rd_a, b) if b < NGA else (rd_b, b - NGA)
        nc.vector.reduce_sum(rd[:, g:g + 1], dt, axis=AX.X)
        # strictly-upper strip copies (DVE writes the f32r tiles)
        if b < 4:
            u0 = (b + 1) * 128
            copied_to = u0
            for (c0, g_ps) in g_pss:
                c1 = c0 + g_ps.shape[1]
                lo = max(copied_to, c0)
                if c1 <= lo:
                    continue
                nc.vector.tensor_copy(grow[b][:, lo - u0:c1 - u0],
                                      g_ps[:, lo - c0:c1 - c0])
                copied_to = c1
        if b == NGA - 1:
            emit_group_entry(0)
        if b == NB - 1:
            emit_group_entry(1)

    def goff(k, i):  # G[k-block, i-block] as lhsT, i > k
        return grow[k][:, (i - k - 1) * 128:(i - k) * 128]

    for it in range(NEWTON):
        m1a = psum.tile([128, WA], FP, tag="y0", name=f"m1a{it}")
        m1b = psum.tile([128, WB], FP, tag="y1", name=f"m1b{it}")
        for b in range(NGA):
            nc.tensor.matmul(m1a[:, _bs(b)], lhsT=rta[:, _bs(b)],
                             rhs=xa[:, _bs(b)], start=True, stop=True)
        for b in range(NB - NGA):
            nc.tensor.matmul(m1b[:, _bs(b)], lhsT=rtb[:, _bs(b)],
                             rhs=xb2[:, _bs(b)], start=True, stop=True)
        m2a = work.tile([128, WA], BF, tag="m2a")
        nc.vector.scalar_tensor_tensor(
            out=m2a, in0=m1a, scalar=-1.0, in1=eye2all[:, 0:WA],
            op0=OP.mult, op1=OP.add)
        m2b = work.tile([128, WB], BF, tag="m2b")
        nc.vector.scalar_tensor_tensor(
            out=m2b, in0=m1b, scalar=-1.0, in1=eye2all[:, WA:NP],
            op0=OP.mult, op1=OP.add)
        xna = psum.tile([128, WA], FP, tag="y2", name=f"xna{it}")
        cna = psum.tile([128, WA], FP, tag="y4", name=f"cna{it}")
        for b in range(NGA):
            nc.tensor.matmul(xna[:, _bs(b)], lhsT=ca[:, _bs(b)],
                             rhs=m2a[:, _bs(b)], start=True, stop=True)
            nc.tensor.matmul(cna[:, _bs(b)], lhsT=m2a[:, _bs(b)],
                             rhs=ca[:, _bs(b)], start=True, stop=True)
        xnb = psum.tile([128, WB], FP, tag="y3", name=f"xnb{it}")
        cnb = psum.tile([128, WB], FP, tag="y5", name=f"cnb{it}")
        for b in range(NB - NGA):
            nc.tensor.matmul(xnb[:, _bs(b)], lhsT=cb[:, _bs(b)],
                             rhs=m2b[:, _bs(b)], start=True, stop=True)
            nc.tensor.matmul(cnb[:, _bs(b)], lhsT=m2b[:, _bs(b)],
                             rhs=cb[:, _bs(b)], start=True, stop=True)
        xa = work.tile([128, WA], BF, tag="xa")
        nc.scalar.copy(xa, xna)
        ca = work.tile([128, WA], BF, tag="ca")
        nc.vector.tensor_copy(ca, cna)
        xb2 = work.tile([128, WB], BF, tag="xb2")
        nc.scalar.copy(xb2, xnb)
        cb = work.tile([128, WB], BF, tag="cb")
        nc.scalar.copy(cb, cnb)

    # f32r copies of the converged diagonal inverse (and its transpose)
    xfa = consts.tile([128, WA], FPR, tag="xfa")
    nc.vector.tensor_copy(xfa, xa)
    xfb = consts.tile([128, WB], FPR, tag="xfb")
    nc.vector.tensor_copy(xfb, xb2)
    cfa = consts.tile([128, WA], FPR, tag="cfa")
    nc.vector.tensor_copy(cfa, ca)
    cfb = consts.tile([128, WB], FPR, tag="cfb")
    nc.vector.tensor_copy(cfb, cb)

    def xdiag(b):
        return xfa[:, _bs(b)] if b < NGA else xfb[:, _bs(b - NGA)]

    def cdiag(b):
        return cfa[:, _bs(b)] if b < NGA else cfb[:, _bs(b - NGA)]

    # zero-padded X rows for the wide back-substitution (cols j = 0..3).
    # Only the regions read before written need pre-zeroing: row k's cols
    # (k+1)*128..512, for k <= 2.  Zeroed via DVE multiply-by-zero
    # (gpsimd memset can't write f32r).
    xfull = []
    for k in range(NB):
        t = consts.tile([128, 512], FPR, tag=f"xf{k}", name=f"xf{k}")
        if k <= 2:
            z0 = (k + 1) * 128
            nc.vector.tensor_scalar_mul(t[:, z0:512], vna[0][:, 0:512 - z0],
                                        0.0)
        xfull.append(t)
    for k in range(4):
        nc.vector.tensor_copy(xfull[k][:, _bs(k)], xdiag(k))

    # --- back-substitution + progressive WT accumulation ---
    # step i: X_i,(0:i) = -X_ii * sum_{k<i} G_ki^T Xrow_k[0:i*128]
    # (rows are zero-padded above the diagonal, so one wide matmul per k).
    # After each row i is final, its WT terms  wt_ps[j] += X_ij^T vna_i
    # accumulate immediately (interleaved PSUM groups on separate banks).
    wt_ps = [psum.tile([128, S], FP, tag=f"y{j}", name=f"wtp{j}")
             for j in range(NB)]

    def emit_wt_terms(i):
        for j in range(i + 1):
            z = xdiag(i) if j == i else xfull[i][:, _bs(j)]
            nc.tensor.matmul(wt_ps[j], lhsT=z, rhs=vna[i],
                             start=(i == j), stop=(i == NB - 1),
                             skip_group_check=True)

    emit_wt_terms(0)
    for i in range(1, NB):
        w = i * 128
        acc_ps = psum.tile([128, w], FP, tag="y5", name=f"acc{i}")
        for k in range(i):
            nc.tensor.matmul(acc_ps, lhsT=goff(k, i), rhs=xfull[k][:, 0:w],
                             start=(k == 0), stop=(k == i - 1))
        nacc = work.tile([128, w], FPR, tag="nacc")
        nc.vector.tensor_scalar_mul(nacc, acc_ps, -1.0)
        xij_ps = psum.tile([128, w], FP, tag="y6", name=f"xij{i}")
        nc.tensor.matmul(xij_ps, lhsT=cdiag(i), rhs=nacc,
                         start=True, stop=True)
        nc.vector.tensor_copy(xfull[i][:, 0:w], xij_ps)
        emit_wt_terms(i)

    wt_sb = []
    for j in range(NB):
        wt = consts.tile([128, S], FPR, tag=f"wt{j}", name=f"wt{j}")
        nc.vector.tensor_copy(wt, wt_ps[j])
        wt_sb.append(wt)

    # --- A = I - WT^T vnat, cast to bf16 (4 tiles [128(s), 512(c)]) ---
    a_bf = []
    for st in range(4):
        a_ps = ptile([128, S], f"a{st}")
        for j in range(NB):
            nc.tensor.matmul(a_ps, lhsT=wt_sb[j][:, _bs(st)], rhs=vna[j],
                             start=(j == 0), stop=(j == NB - 1))
        a = consts.tile([128, S], BF, tag=f"a{st}", name=f"a{st}")
        nc.scalar.mul(a, a_ps, -1.0)
        nc.vector.tensor_add(a[:, _bs(st)], a[:, _bs(st)], eye_bf)
        a_bf.append(a)
    return a_bf


def build_program(trace_sim=False):
    nc = bass.Bass("TRN2")
    xt_d = nc.dram_tensor("xt", [S, BPC], BF, kind="ExternalInput")
    vt_d = nc.dram_tensor("vt", [S, NP], FPR, kind="ExternalInput")
    vnat_d = nc.dram_tensor("vnat", [NP, S], FPR, kind="ExternalInput")
    y_d = nc.dram_tensor("y", [S, BPC], BF, kind="ExternalOutput")

    with tile.TileContext(nc, trace_sim=trace_sim) as tc, ExitStack() as ctx:
        consts = ctx.enter_context(tc.tile_pool(name="consts", bufs=1))
        work = ctx.enter_context(tc.tile_pool(name="work", bufs=2))
        ypool = ctx.enter_context(tc.tile_pool(name="ypool", bufs=3))
        psum = ctx.enter_context(
            tc.tile_pool(name="psum", bufs=1, space="PSUM"))

        # x^T resident in SBUF (8 MB bf16)
        xb = [consts.tile([128, BPC], BF, tag=f"xb{k}", name=f"xb{k}")
              for k in range(4)]

        a_bf = _emit_prologue(nc, vt_d, vnat_d, xt_d, xb, consts, work, psum)

        # --- main loop: y^T[c,r] = sum_k A[k-strip, c-strip]^T x^T[k, r] ---
        nmm = 0
        for c in range(4):
            for rg in range(BPC // RG):
                yt = ypool.tile([128, RG], BF, tag="yt")
                for r in range(RG // RW):
                    y_ps = psum.tile([128, RW], FP, tag=f"y{nmm % 7}")
                    nmm += 1
                    r0 = rg * RG + r * RW
                    for k in range(4):
                        nc.tensor.matmul(
                            y_ps,
                            lhsT=a_bf[k][:, _bs(c)],
                            rhs=xb[k][:, r0:r0 + RW],
                            start=(k == 0), stop=(k == 3))
                    if r % 2 == 0:
                        nc.scalar.copy(yt[:, r * RW:(r + 1) * RW], y_ps)
                    else:
                        nc.vector.tensor_copy(yt[:, r * RW:(r + 1) * RW], y_ps)
                nc.sync.dma_start(
                    out=y_d[_bs(c), rg * RG:(rg + 1) * RG], in_=yt)
    _split_excess_waits(nc)
    return nc


_NC_CACHE = {}


def _get_nc():
    if "nc" not in _NC_CACHE:
        _NC_CACHE["nc"] = build_program()
    return _NC_CACHE["nc"]


def prepare_in_maps(x, vectors):
    x = np.asarray(x, dtype=np.float32)
    v = np.asarray(vectors, dtype=np.float32)[..., 0]  # [514, 512]
    vnat = np.zeros((NP, S), np.float32)
    vnat[:NV] = v
    vt = np.ascontiguousarray(vnat.T)                  # [512, 640]
    xbf = x.astype(ml_dtypes.bfloat16)                 # [65536, 512] bf16
    xt = np.ascontiguousarray(xbf.T)                   # [512, 65536] bf16
    in_maps = []
    for c in range(NCORES):
        in_maps.append({
            "xt": np.ascontiguousarray(xt[:, c * BPC:(c + 1) * BPC]),
            "vt": vt,
            "vnat": vnat,
        })
    return in_maps


def finish_output(res):
    yt = np.concatenate([r["y"] for r in res.results], axis=1)  # [512, 65536]
    y = yt.T.astype(np.float32)                                 # [65536, 512]
    return np.ascontiguousarray(y)


def kernel(x, vectors):
    nc = _get_nc()
    in_maps = prepare_in_maps(x, vectors)
    res = run_bass_kernel_spmd(nc, in_maps, list(range(NCORES)))
    return finish_output(res)


if __name__ == "__main__":
    rng = np.random.default_rng(0)
    x = rng.standard_normal((B, S)).astype(np.float32)
    v = rng.standard_normal((NV, S, 1)).astype(np.float32)
    v /= np.linalg.norm(v, axis=1, keepdims=True)
    y = kernel(x, v)
    print("y", y.shape, y.dtype, float(np.abs(y).max()))


# revision 16
# speedup vs baseline: 2.3852x; 1.0367x over previous
"""Trainium2 Bass kernel for the Householder-chain problem.

Computes y = x @ Q.T where Q = M_0 @ M_1 @ ... @ M_{N-1} is a product of
N=514 Householder reflections M_i = I - 2 v_i v_i^T / (v_i^T v_i + eps)
over S=512 dims, and x is [65536, 512].

Math: since each M_i is symmetric, Q.T = M_{N-1} @ ... @ M_0 =: A, and the
product collapses via the compact-WY representation with natural column
order:  A = I - V T V^T  where V = [v_0 ... v_{N-1}] (S x N) and
T^{-1} = R = stril(V^T V) + diag((||v_i||^2 + eps)/2)   (lower triangular).

On device (replicated on each of 8 cores, since it is tiny):
  G = V^T V (f32r row-strip matmuls, all moving dims >= 256 so the PE runs
  at full rate); the five 128x128 diagonal blocks of R are inverted by
  Newton iteration X <- X(2I - R X) in bf16, run as two independent
  dependency chains (blocks 0-2 and 3-4) so engine latencies overlap.
  Off-diagonal blocks of X = R^{-1} come from a zero-padded full-row
  back-substitution in f32r (one wide accumulating matmul per block-row
  instead of per-(i,j) 128-wide matmuls); each back-substitution step
  feeds its row's terms of WT = (V T)^T into per-column PSUM accumulators
  immediately, then A = I - WT^T V, cast to bf16.  N is zero-padded
  514 -> 640 with unit diagonal entries in R for the pad columns, which
  leaves A unchanged.

Main work: y = x @ A, data-parallel over the 65536 rows across 8 cores
(8192 rows/core).  It runs weight-stationary in bf16 producing y^T tiles:
out[c, r] = sum_k A[k-strip, c-strip]^T x^T[k-strip, r], with x^T uploaded
in bf16 (8 MB/core, fully resident in SBUF; the DMAs are issued first and
stream in behind the small v loads while the PE runs the prologue) and
y^T stored in bf16.  The host un-transposes and casts back to float32.
End-to-end relative error ~4e-3 (gate is 2e-2).
"""

from contextlib import ExitStack

import numpy as np
import ml_dtypes

import bass_rust
import concourse.bass as bass
import concourse.mybir as mybir
import concourse.tile as tile
from concourse.bass_utils import run_bass_kernel_spmd
from concourse.masks import make_identity, make_upper_triangular
from concourse.vector_clock import ScopedClock

FP = mybir.dt.float32
FPR = mybir.dt.float32r
BF = mybir.dt.bfloat16
AX = mybir.AxisListType
OP = mybir.AluOpType

S = 512           # feature dim
NV = 514          # number of householder vectors
NP = 640          # padded vector count (5 * 128)
NB = NP // 128    # 5 blocks
B = 65536         # batch rows
NCORES = 8
BPC = B // NCORES  # 8192 rows per core
EPS = 1e-16
NEWTON = 4        # bf16 Newton iterations (converged; see numerics note)
RW = 512          # main-loop r-block width (moving free dim)
RG = 2048         # store-group width (columns per output DMA)
NGA = 3           # Newton group a: blocks 0..2
WA, WB = NGA * 128, (NB - NGA) * 128


# ---------------------------------------------------------------------------
# walrus CTRL instructions accept at most 4 sem waits, and this Tile
# version puts the whole global-clock wait set on the single tail drain.
# Spread the waits over preceding SP nops (1 wait each, conservatively).
def _patched_drain_and_barrier(self, tick_clock, wait_clock):
    pre_nops = [self.nc.sync.nop() for _ in range(30)]
    drain_inst = self.nc.sync.drain()
    wait_clock.add_sem_waits(
        drain_inst.ins, ScopedClock({None: tick_clock.global_clock})
    )
    si = drain_inst.ins.sync_info
    waits = list(si.on_wait) if si is not None and si.on_wait else []
    if len(waits) > 1:
        assert len(waits) - 1 <= len(pre_nops), "too many drain waits"
        for nop, w in zip(pre_nops, waits[:-1]):
            nop.ins.sync_info = bass_rust.SyncInfo(on_wait=[w], on_update=[])
        upd = list(si.on_update) if si.on_update else []
        drain_inst.ins.sync_info = bass_rust.SyncInfo(
            on_wait=[waits[-1]], on_update=upd)

    self.nc.all_engine_barrier()
    assert self.sems is not None
    popped = self.nc._tile_sem_poison_stack.pop()
    assert popped is self._sem_poison
    # clear_and_free_semaphores, but issuing the dma_reset/sem_clear from
    # the Sync engine instead of GpSimd — gpsimd dispatch carries ~2us of
    # fixed overhead per op, which put ~4us of dead time in the kernel
    # tail barrier.
    sems = list(self.sems.allocated().values())
    if sems:
        sem_nums = [s.num if hasattr(s, "num") else s for s in sems]
        for r in bass.compact_to_ranges(sem_nums):
            assert self.nc._state.free_isdisjoint(r)
            self.nc.sync.drain(semaphore_range=r)
            self.nc.sync.sem_clear(r)
        self.nc._state.prepend_free_semaphores(sem_nums)
        for ps in self.nc._tile_sem_poison_stack:
            ps.update(sem_nums)
    self.nc.all_engine_barrier()


tile.TileContext._drain_and_barrier = _patched_drain_and_barrier


def _split_excess_waits(nc, max_waits=1):
    """This walrus build accepts very few sem waits per instruction (a
    TensorTensor with 2 was rejected).  Hoist all but `max_waits` of each
    instruction's waits onto same-engine NOPs inserted right before it —
    engines execute in order, so semantics are unchanged."""
    idx = 0
    for fn in nc.m.functions:
        for bb in fn.blocks:
            new = []
            changed = False
            for inst in bb.instructions:
                si = inst.sync_info
                waits = list(si.on_wait) if si is not None and si.on_wait else []
                if len(waits) > max_waits:
                    changed = True
                    for w in waits[:-max_waits]:
                        idx += 1
                        nop = mybir.InstNoOp(
                            name=f"I-waitsplit-{idx}", engine=inst.engine)
                        nop.sync_info = bass_rust.SyncInfo(
                            on_wait=[w], on_update=[])
                        new.append(nop)
                    upd = list(si.on_update) if si.on_update else []
                    inst.sync_info = bass_rust.SyncInfo(
                        on_wait=waits[-max_waits:], on_update=upd)
                new.append(inst)
            if changed:
                bb.instructions = new
# ---------------------------------------------------------------------------


def _bs(b):
    return slice(b * 128, (b + 1) * 128)


def _emit_prologue(nc, vt_d, vnat_d, xt_d, xb, consts, work, psum):
    """Emit instructions computing A as 4 bf16 sbuf tiles [128(s), 512(c)].
    Issues all input DMAs first (v loads, then the big x^T loads)."""
    ptag = [0]

    def ptile(shape, name):  # rotating psum allocator (tags y0..y6)
        t = psum.tile(shape, FP, tag=f"y{ptag[0] % 7}", name=name)
        ptag[0] += 1
        return t

    # --- input DMAs first: v loads gate the prologue; x^T streams behind ---
    vtr = []
    for k in range(4):
        t = consts.tile([128, NP], FPR, tag=f"vt{k}", name=f"vt{k}")
        nc.sync.dma_start(out=t, in_=vt_d[_bs(k), :])
        vtr.append(t)
    vna = []
    for j in range(NB):
        t = consts.tile([128, S], FPR, tag=f"vnat{j}", name=f"vnat{j}")
        nc.sync.dma_start(out=t, in_=vnat_d[_bs(j), :])
        vna.append(t)
    for k in range(4):
        nc.sync.dma_start(out=xb[k], in_=xt_d[_bs(k), :])

    # --- mask constants (GpSimd/DVE, overlap the DMAs) ---
    eye = consts.tile([128, 128], FP, tag="eye")
    make_identity(nc, eye)
    triu = consts.tile([128, 128], FP, tag="triu")
    make_upper_triangular(nc, triu, val=1.0, diag=False)
    padcol = consts.tile([128, 1], FP, tag="padcol")
    nc.gpsimd.memset(padcol, 1.0)
    nc.gpsimd.affine_select(
        out=padcol, in_=padcol, compare_op=OP.is_ge, fill=0.0,
        base=-(NV - 4 * 128), pattern=[[0, 1]], channel_multiplier=1,
    )
    eye2all = consts.tile([128, NP], FP, tag="eye2all")
    for b in range(NB):
        nc.gpsimd.tensor_scalar_mul(eye2all[:, _bs(b)], eye, 2.0)
    eye_bf = consts.tile([128, 128], BF, tag="eye_bf")
    nc.gpsimd.tensor_copy(eye_bf, eye)
    # --- G = V^T V in f32r, wide row strips (moving dim >= 256), fused
    # with the per-group rd/RT/X0 chains: group a (blocks 0-2) has its
    # whole Newton-entry chain emitted right after block 2's strip, so it
    # runs on DVE while the PE is still doing blocks 3-4 of G. ---
    chunks = {0: [(0, 512), (384, 640)], 1: [(128, 640)], 2: [(256, 640)],
              3: [(384, 640)], 4: [(384, 640)]}
    gd = []      # diagonal blocks [128,128] fp32
    grow = []    # strictly-upper row strips, f32r (cols re-based)
    grow_w = [512, 384, 256, 128]
    for b in range(NB - 1):
        grow.append(consts.tile([128, grow_w[b]], FPR, tag=f"g{b}",
                                name=f"g{b}"))
    rd_a = consts.tile([128, NGA], FP, tag="rd_a")
    rd_b = consts.tile([128, NB - NGA], FP, tag="rd_b")
    ri_a = consts.tile([128, NGA], FP, tag="ri_a")
    ri_b = consts.tile([128, NB - NGA], FP, tag="ri_b")
    rta = consts.tile([128, WA], BF, tag="rta")
    rtb = consts.tile([128, WB], BF, tag="rtb")
    xa = work.tile([128, WA], BF, tag="xa")
    xb2 = work.tile([128, WB], BF, tag="xb2")
    ca = work.tile([128, WA], BF, tag="ca")
    cb = work.tile([128, WB], BF, tag="cb")

    def rt_slice(b):
        return rta[:, _bs(b)] if b < NGA else rtb[:, _bs(b - NGA)]

    def emit_group_entry(grp):
        # (rd+eps)/2 [+1 pad], reciprocal, RT build, X0=C0=diag(1/rd)
        rd, ri = (rd_a, ri_a) if grp == 0 else (rd_b, ri_b)
        blocks = range(NGA) if grp == 0 else range(NGA, NB)
        nc.vector.tensor_scalar(rd, rd, EPS, 0.5, OP.add, OP.mult)
        if grp == 1:
            nc.vector.tensor_add(rd[:, -1:], rd[:, -1:], padcol)
        nc.vector.reciprocal(ri, rd)
        for b in blocks:
            g = b if grp == 0 else b - NGA
            rtm = work.tile([128, 128], FP, tag="rtm")
            nc.vector.tensor_mul(rtm, gd[b], triu)
            nc.vector.scalar_tensor_tensor(
                out=rt_slice(b), in0=eye, scalar=rd[:, g:g + 1],
                in1=rtm, op0=OP.mult, op1=OP.add)
            dst = xa[:, _bs(b)] if grp == 0 else xb2[:, _bs(b - NGA)]
            nc.scalar.activation(dst, eye, mybir.ActivationFunctionType.Copy,
                                 scale=ri[:, g:g + 1])
        if grp == 0:
            nc.scalar.copy(ca, xa)
        else:
            nc.scalar.copy(cb, xb2)

    gtag = [0]
    for b in range(NB):
        g_pss = []
        for (c0, c1) in chunks[b]:
            g_ps = psum.tile([128, c1 - c0], FP, tag=f"y{gtag[0] % 6}",
                             name=f"g{b}_{c0}")
            gtag[0] += 1
            for k in range(4):
                nc.tensor.matmul(g_ps, lhsT=vtr[k][:, _bs(b)],
                                 rhs=vtr[k][:, c0:c1],
                                 start=(k == 0), stop=(k == 3))
            g_pss.append((c0, g_ps))
        c0d, psd = g_pss[0] if b < 4 else g_pss[-1]
        gdb = consts.tile([128, 128], FP, tag=f"gd{b}", name=f"gd{b}")
        nc.scalar.copy(gdb, psd[:, b * 128 - c0d:(b + 1) * 128 - c0d])
        gd.append(gdb)
        # rd terms for this block (DVE, overlaps later G matmuls)
        dt = work.tile([128, 128], FP, tag="dtmp")
        nc.vector.tensor_mul(dt, gdb, eye)
        rd, g = (rd_a, b) if b < NGA else (rd_b, b - NGA)
        nc.vector.reduce_sum(rd[:, g:g + 1], dt, axis=AX.X)
        # strictly-upper strip copies (DVE writes the f32r tiles)
        if b < 4:
            u0 = (b + 1) * 128
            copied_to = u0
            for (c0, g_ps) in g_pss:
                c1 = c0 + g_ps.shape[1]
                lo = max(copied_to, c0)
                if c1 <= lo:
                    continue
                nc.vector.tensor_copy(grow[b][:, lo - u0:c1 - u0],
                                      g_ps[:, lo - c0:c1 - c0])
                copied_to = c1
        if b == NGA - 1:
            emit_group_entry(0)
        if b == NB - 1:
            emit_group_entry(1)

    def goff(k, i):  # G[k-block, i-block] as lhsT, i > k
        return grow[k][:, (i - k - 1) * 128:(i - k) * 128]

    for it in range(NEWTON):
        m1a = psum.tile([128, WA], FP, tag="y0", name=f"m1a{it}")
        m1b = psum.tile([128, WB], FP, tag="y1", name=f"m1b{it}")
        for b in range(NGA):
            nc.tensor.matmul(m1a[:, _bs(b)], lhsT=rta[:, _bs(b)],
                             rhs=xa[:, _bs(b)], start=True, stop=True)
        for b in range(NB - NGA):
            nc.tensor.matmul(m1b[:, _bs(b)], lhsT=rtb[:, _bs(b)],
                             rhs=xb2[:, _bs(b)], start=True, stop=True)
        m2a = work.tile([128, WA], BF, tag="m2a")
        nc.vector.scalar_tensor_tensor(
            out=m2a, in0=m1a, scalar=-1.0, in1=eye2all[:, 0:WA],
            op0=OP.mult, op1=OP.add)
        m2b = work.tile([128, WB], BF, tag="m2b")
        nc.vector.scalar_tensor_tensor(
            out=m2b, in0=m1b, scalar=-1.0, in1=eye2all[:, WA:NP],
            op0=OP.mult, op1=OP.add)
        xna = psum.tile([128, WA], FP, tag="y2", name=f"xna{it}")
        cna = psum.tile([128, WA], FP, tag="y4", name=f"cna{it}")
        for b in range(NGA):
            nc.tensor.matmul(xna[:, _bs(b)], lhsT=ca[:, _bs(b)],
                             rhs=m2a[:, _bs(b)], start=True, stop=True)
            nc.tensor.matmul(cna[:, _bs(b)], lhsT=m2a[:, _bs(b)],
                             rhs=ca[:, _bs(b)], start=True, stop=True)
        xnb = psum.tile([128, WB], FP, tag="y3", name=f"xnb{it}")
        cnb = psum.tile([128, WB], FP, tag="y5", name=f"cnb{it}")
        for b in range(NB - NGA):
            nc.tensor.matmul(xnb[:, _bs(b)], lhsT=cb[:, _bs(b)],
                             rhs=m2b[:, _bs(b)], start=True, stop=True)
            nc.tensor.matmul(cnb[:, _bs(b)], lhsT=m2b[:, _bs(b)],
                             rhs=cb[:, _bs(b)], start=True, stop=True)
        xa = work.tile([128, WA], BF, tag="xa")
        nc.scalar.copy(xa, xna)
        ca = work.tile([128, WA], BF, tag="ca")
        nc.vector.tensor_copy(ca, cna)
        xb2 = work.tile([128, WB], BF, tag="xb2")
        nc.scalar.copy(xb2, xnb)
        cb = work.tile([128, WB], BF, tag="cb")
        nc.scalar.copy(cb, cnb)

    # f32r copies of the converged diagonal inverse (and its transpose)
    xfa = consts.tile([128, WA], FPR, tag="xfa")
    nc.vector.tensor_copy(xfa, xa)
    xfb = consts.tile([128, WB], FPR, tag="xfb")
    nc.vector.tensor_copy(xfb, xb2)
    cfa = consts.tile([128, WA], FPR, tag="cfa")
    nc.vector.tensor_copy(cfa, ca)
    cfb = consts.tile([128, WB], FPR, tag="cfb")
    nc.vector.tensor_copy(cfb, cb)

    def xdiag(b):
        return xfa[:, _bs(b)] if b < NGA else xfb[:, _bs(b - NGA)]

    def cdiag(b):
        return cfa[:, _bs(b)] if b < NGA else cfb[:, _bs(b - NGA)]

    # zero-padded X rows for the wide back-substitution (cols j = 0..3).
    # Only the regions read before written need pre-zeroing: row k's cols
    # (k+1)*128..512, for k <= 2.  Zeroed via DVE multiply-by-zero
    # (gpsimd memset can't write f32r).
    xfull = []
    for k in range(NB):
        t = consts.tile([128, 512], FPR, tag=f"xf{k}", name=f"xf{k}")
        if k <= 2:
            z0 = (k + 1) * 128
            nc.vector.tensor_scalar_mul(t[:, z0:512], vna[0][:, 0:512 - z0],
                                        0.0)
        xfull.append(t)
    for k in range(4):
        nc.vector.tensor_copy(xfull[k][:, _bs(k)], xdiag(k))

    # --- back-substitution + progressive WT accumulation ---
    # step i: X_i,(0:i) = -X_ii * sum_{k<i} G_ki^T Xrow_k[0:i*128]
    # (rows are zero-padded above the diagonal, so one wide matmul per k).
    # After each row i is final, its WT terms  wt_ps[j] += X_ij^T vna_i
    # accumulate immediately (interleaved PSUM groups on separate banks).
    wt_ps = [psum.tile([128, S], FP, tag=f"y{j}", name=f"wtp{j}")
             for j in range(NB)]

    def emit_wt_terms(i):
        for j in range(i + 1):
            z = xdiag(i) if j == i else xfull[i][:, _bs(j)]
            nc.tensor.matmul(wt_ps[j], lhsT=z, rhs=vna[i],
                             start=(i == j), stop=(i == NB - 1),
                             skip_group_check=True)

    emit_wt_terms(0)
    for i in range(1, NB):
        w = i * 128
        acc_ps = psum.tile([128, w], FP, tag="y5", name=f"acc{i}")
        for k in range(i):
            nc.tensor.matmul(acc_ps, lhsT=goff(k, i), rhs=xfull[k][:, 0:w],
                             start=(k == 0), stop=(k == i - 1))
        nacc = work.tile([128, w], FPR, tag="nacc")
        nc.vector.tensor_scalar_mul(nacc, acc_ps, -1.0)
        xij_ps = psum.tile([128, w], FP, tag="y6", name=f"xij{i}")
        nc.tensor.matmul(xij_ps, lhsT=cdiag(i), rhs=nacc,
                         start=True, stop=True)
        nc.vector.tensor_copy(xfull[i][:, 0:w], xij_ps)
        emit_wt_terms(i)

    wt_sb = []
    for j in range(NB):
        wt = consts.tile([128, S], FPR, tag=f"wt{j}", name=f"wt{j}")
        nc.vector.tensor_copy(wt, wt_ps[j])
        wt_sb.append(wt)

    # --- A = I - WT^T vnat, cast to bf16 (4 tiles [128(s), 512(c)]) ---
    a_bf = []
    for st in range(4):
        a_ps = ptile([128, S], f"a{st}")
        for j in range(NB):
            nc.tensor.matmul(a_ps, lhsT=wt_sb[j][:, _bs(st)], rhs=vna[j],
                             start=(j == 0), stop=(j == NB - 1))
        a = consts.tile([128, S], BF, tag=f"a{st}", name=f"a{st}")
        nc.scalar.mul(a, a_ps, -1.0)
        nc.vector.tensor_add(a[:, _bs(st)], a[:, _bs(st)], eye_bf)
        a_bf.append(a)
    return a_bf


def build_program(trace_sim=False):
    nc = bass.Bass("TRN2")
    xt_d = nc.dram_tensor("xt", [S, BPC], BF, kind="ExternalInput")
    vt_d = nc.dram_tensor("vt", [S, NP], FPR, kind="ExternalInput")
    vnat_d = nc.dram_tensor("vnat", [NP, S], FPR, kind="ExternalInput")
    y_d = nc.dram_tensor("y", [S, BPC], BF, kind="ExternalOutput")

    with tile.TileContext(nc, trace_sim=trace_sim) as tc, ExitStack() as ctx:
        consts = ctx.enter_context(tc.tile_pool(name="consts", bufs=1))
        work = ctx.enter_context(tc.tile_pool(name="work", bufs=2))
        ypool = ctx.enter_context(tc.tile_pool(name="ypool", bufs=3))
        psum = ctx.enter_context(
            tc.tile_pool(name="psum", bufs=1, space="PSUM"))

        # x^T resident in SBUF (8 MB bf16)
        xb = [consts.tile([128, BPC], BF, tag=f"xb{k}", name=f"xb{k}")
              for k in range(4)]

        a_bf = _emit_prologue(nc, vt_d, vnat_d, xt_d, xb, consts, work, psum)

        # --- main loop: y^T[c,r] = sum_k A[k-strip, c-strip]^T x^T[k, r] ---
        nmm = 0
        for c in range(4):
            for rg in range(BPC // RG):
                yt = ypool.tile([128, RG], BF, tag="yt")
                for r in range(RG // RW):
                    y_ps = psum.tile([128, RW], FP, tag=f"y{nmm % 7}")
                    nmm += 1
                    r0 = rg * RG + r * RW
                    for k in range(4):
                        nc.tensor.matmul(
                            y_ps,
                            lhsT=a_bf[k][:, _bs(c)],
                            rhs=xb[k][:, r0:r0 + RW],
                            start=(k == 0), stop=(k == 3))
                    if r % 2 == 0:
                        nc.scalar.copy(yt[:, r * RW:(r + 1) * RW], y_ps)
                    else:
                        nc.vector.tensor_copy(yt[:, r * RW:(r + 1) * RW], y_ps)
                nc.sync.dma_start(
                    out=y_d[_bs(c), rg * RG:(rg + 1) * RG], in_=yt)
    _split_excess_waits(nc)
    return nc


_NC_CACHE = {}


def _get_nc():
    if "nc" not in _NC_CACHE:
        _NC_CACHE["nc"] = build_program()
    return _NC_CACHE["nc"]


def prepare_in_maps(x, vectors):
    x = np.asarray(x, dtype=np.float32)
    v = np.asarray(vectors, dtype=np.float32)[..., 0]  # [514, 512]
    vnat = np.zeros((NP, S), np.float32)
    vnat[:NV] = v
    vt = np.ascontiguousarray(vnat.T)                  # [512, 640]
    xbf = x.astype(ml_dtypes.bfloat16)                 # [65536, 512] bf16
    xt = np.ascontiguousarray(xbf.T)                   # [512, 65536] bf16
    in_maps = []
    for c in range(NCORES):
        in_maps.append({
            "xt": np.ascontiguousarray(xt[:, c * BPC:(c + 1) * BPC]),
            "vt": vt,
            "vnat": vnat,
        })
    return in_maps


def finish_output(res):
    yt = np.concatenate([r["y"] for r in res.results], axis=1)  # [512, 65536]
    y = yt.T.astype(np.float32)                                 # [65536, 512]
    return np.ascontiguousarray(y)


def kernel(x, vectors):
    nc = _get_nc()
    in_maps = prepare_in_maps(x, vectors)
    res = run_bass_kernel_spmd(nc, in_maps, list(range(NCORES)))
    return finish_output(res)


if __name__ == "__main__":
    rng = np.random.default_rng(0)
    x = rng.standard_normal((B, S)).astype(np.float32)
    v = rng.standard_normal((NV, S, 1)).astype(np.float32)
    v /= np.linalg.norm(v, axis=1, keepdims=True)
    y = kernel(x, v)
    print("y", y.shape, y.dtype, float(np.abs(y).max()))
